# revision 30
# baseline (speedup 1.0000x reference)
"""Trainium2 Bass kernel for nn_Attention_41996190220419.

Single-head causal attention with softplus weights and a time-flipped
rotary embedding, B=8 T=2048 C=1024 fp32.

Sharding: pure data-parallel over batch (1 batch element per NeuronCore,
8 cores, no collectives).

Optimizations over the 413us baseline (all per-core):
  * Wv/Wp folding: out = (wei @ x) @ (Wv Wp) + rowsum(wei) x (bv Wp) + bp.
    The V GEMM (256 N=512 matmuls) disappears; wei@x consumes x in its
    native [t, c] layout (the same tiles the transposes read), and the
    rank-1 rowsum term is PE-cheap: per span one M=1 accumulation chain
    over the masked score tiles plus one K=1 float32r matmul appended to
    each projection accumulation group.
  * Scores in fp8 e4m3 with MatmulPerfMode.DoubleRow (2 contraction rows
    per PE cell): the rotated K/Q are written by the DVE straight into
    [128, 2, T] paired-group tiles; each score tile is 2 halves x 4
    DoubleRow matmuls (K=256 each) instead of 8 bf16 N=512 matmuls.
    Numerically validated: L2 rel err ~6.6e-3 (budget 2e-2).
  * Causal skip: diagonal-block score halves that are fully above the
    diagonal (d>=2, left half) are never computed (gpsimd memset zeroes
    the st half instead).
  * Rotation arithmetic in bf16 (2x DVE throughput); error is far below
    the fp8 quantization already applied to the rotated K/Q.

Per-core phases (matmuls bf16 with fp32 PSUM accumulate unless noted):
  0: x [T,C] bf16 -> 16 resident xs tiles; XT via PE transposes
  1: KT/QT = W^T XT (+bias via ACT), bf16 rotation on DVE -> fp8 pair
     tiles kr8/qr8 [128, 2, T]
  2: per 512-span: ST[j,i] via fp8 DoubleRow (halves of 256), softplus =
     Ln(Exp(x)+1) on ACT, diagonal masks on DVE, rowsum via M=1 chain
  3: OT[c,i] = sum_j x[j,c] ST[j,i] (PSUM accumulate over j, bf16)
  4: OUT[t,:] = sum_c OT[c,t] Wvp[c,:] + rowsum*bvp (K=1 f32r matmul in
     the same PSUM group) + bp -> DRAM

The even/odd rotation pairs are tile-level structure: Wk/Wq columns (and
bk/bq) are host-permuted to [evens|odds]; scores are invariant to any
channel permutation applied to both K and Q. cos/sin tables, masks and
the identity are host-precomputed inputs.
"""

import os
import sys

if "/opt/trn_rl_repo" not in sys.path:
    sys.path.insert(0, "/opt/trn_rl_repo")

import numpy as np
import ml_dtypes

import concourse.bass as bass
import concourse.bacc as bacc
import concourse.mybir as mybir
import concourse.tile as tile
from concourse.bass_utils import run_bass_kernel_spmd

B, T, C = 8, 2048, 1024
H = C // 2
NCORES = 8
PD = 128
TCH = 512                 # t-chunk width (phase 1) == i-span width (attention)
HF = 256                  # fp8 DoubleRow moving half-width
NT = T // PD              # 16
NSP = T // TCH            # 4
NG = C // PD              # 8
NPR = NG // 2             # 4 fp8 pair tiles
BF16 = mybir.dt.bfloat16
F32 = mybir.dt.float32
F32R = mybir.dt.float32r
F8 = mybir.dt.float8e4
DR = mybir.MatmulPerfMode.DoubleRow
AF = mybir.ActivationFunctionType
INV_SQRT_C = float(C) ** -0.5

_CACHE = {}

LAST_RESULT = None  # BassKernelResults of the most recent run (for profiling)


def _patch_act_tables():
    """Force every ACT func we use (Copy/Identity/Exp/Ln) to resolve to the
    single `natural_log_exp_and_others` table so the Exp/Ln alternation in
    the softplus does not thrash ACT_TABLE_LOADs (1.3us each).
    Table ids are positional, so keep the dict order and only strip
    functions from the other tables."""
    if _CACHE.get("act_patched"):
        return
    from concourse import hw_specs
    orig = hw_specs.get_activation_tables
    combined = "natural_log_exp_and_others"

    def patched(arch):
        tables = orig(arch)
        if combined in tables:
            keep = tables[combined]
            tables = {
                name: (s if name == combined else (s - keep))
                for name, s in tables.items()
            }
        return tables

    hw_specs.get_activation_tables = patched
    bacc.get_activation_tables = patched
    _CACHE["act_patched"] = True


def _build_nc():
    _patch_act_tables()
    nc = bacc.Bacc("TRN2", target_bir_lowering=False, debug=False,
                   num_devices=NCORES)

    x_d = nc.dram_tensor("x", [T, C], BF16, kind="ExternalInput").ap()
    wk_d = nc.dram_tensor("wk", [C, C], BF16, kind="ExternalInput").ap()
    wq_d = nc.dram_tensor("wq", [C, C], BF16, kind="ExternalInput").ap()
    wvp_d = nc.dram_tensor("wvp", [C, C], BF16, kind="ExternalInput").ap()
    bkr_d = nc.dram_tensor("bkr", [PD, NG], F32, kind="ExternalInput").ap()
    bqr_d = nc.dram_tensor("bqr", [PD, NG], F32, kind="ExternalInput").ap()
    bvp_d = nc.dram_tensor("bvp", [1, C], F32R, kind="ExternalInput").ap()
    bpb_d = nc.dram_tensor("bpb", [PD, C], F32, kind="ExternalInput").ap()
    cos_d = nc.dram_tensor("cosT", [H, T], BF16, kind="ExternalInput").ap()
    sin_d = nc.dram_tensor("sinT", [H, T], BF16, kind="ExternalInput").ap()
    msk_d = nc.dram_tensor("masks", [NSP, PD, TCH], BF16,
                           kind="ExternalInput").ap()
    idn_d = nc.dram_tensor("ident", [PD, PD], BF16, kind="ExternalInput").ap()
    out_d = nc.dram_tensor("out", [T, C], F32, kind="ExternalOutput").ap()

    with tile.TileContext(nc) as tc:
        with tc.tile_pool(name="persist", bufs=1) as pp:

            # resident x in native [t, c] layout (transpose source + OT lhsT).
            # DMA queue engines run ~16 GB/s each, so big tiles are split
            # into chunks that land on different queues; the first two x
            # tiles (critical path for the transposes) are split finest.
            xs = [pp.tile([PD, C], BF16, tag=f"xs{j}", name=f"xs{j}")
                  for j in range(NT)]

            def load_xs(j, nchunk, eng=None):
                eng = eng or nc.sync
                w = C // nchunk
                for cc in range(nchunk):
                    eng.dma_start(
                        out=xs[j][:, cc * w:(cc + 1) * w],
                        in_=x_d[j * PD:(j + 1) * PD, cc * w:(cc + 1) * w])

            # first chunk's x tiles split finest and spread across all three
            # DMA rings so the transposes can start ~12us in
            load_xs(0, 4)
            ident = pp.tile([PD, PD], BF16, name="ident")
            nc.sync.dma_start(out=ident, in_=idn_d)
            load_xs(1, 4, nc.scalar)
            load_xs(2, 4, nc.gpsimd)
            load_xs(3, 4)
            # rotated K/Q fp8 pair tiles: slab 0 = even group e, slab 1 = odd
            # group e+4 (DoubleRow contracts both slabs per matmul); split
            # per span so attention span s only depends on chunk-s rotations
            kr8 = [[pp.tile([PD, 2, TCH], F8, tag=f"kr{e}_{s}",
                            name=f"kr{e}_{s}") for s in range(NSP)]
                   for e in range(NPR)]
            qr8 = [[pp.tile([PD, 2, TCH], F8, tag=f"qr{e}_{s}",
                            name=f"qr{e}_{s}") for s in range(NSP)]
                   for e in range(NPR)]

            bkr = pp.tile([PD, NG], F32, name="bkr")
            nc.sync.dma_start(out=bkr, in_=bkr_d)
            bqr = pp.tile([PD, NG], F32, name="bqr")
            nc.sync.dma_start(out=bqr, in_=bqr_d)
            ones = pp.tile([PD, 1], BF16, name="ones")
            nc.gpsimd.memset(ones, 1.0)
            # everything below is needed late; dispatched from gpsimd's DMA
            # ring so the sync ring stays dedicated to x and trig
            mskt = []
            for d in range(NSP):
                m = pp.tile([PD, TCH], BF16, tag=f"msk{d}", name=f"msk{d}")
                mskt.append(m)
            bvp = pp.tile([1, C], F32R, name="bvp")
            bpb = pp.tile([PD, C], F32, name="bpb")
            # masked-score rowsums, one [1, TCH] f32 row per span
            rsum = [pp.tile([1, TCH], F32R, tag=f"rs{s}", name=f"rs{s}")
                    for s in range(NSP)]
            wpsb = [pp.tile([PD, C], BF16, tag=f"wp{ci}", name=f"wp{ci}")
                    for ci in range(NG)]

            def load_late_tensors():
                for ci in range(NG):
                    for cc in range(2):
                        nc.gpsimd.dma_start(
                            out=wpsb[ci][:, cc * 512:(cc + 1) * 512],
                            in_=wvp_d[ci * PD:(ci + 1) * PD,
                                      cc * 512:(cc + 1) * 512])
                for d in range(NSP):
                    nc.gpsimd.dma_start(out=mskt[d], in_=msk_d[d])
                nc.gpsimd.dma_start(out=bvp, in_=bvp_d)
                nc.gpsimd.dma_start(out=bpb, in_=bpb_d)

            # ---------------- phase 0 + 1: XT, rotated K/Q ----------------
            with tc.tile_pool(name="p1", bufs=1) as p1, \
                 tc.tile_pool(name="pstr", bufs=4, space="PSUM") as pstr, \
                 tc.tile_pool(name="psK", bufs=3, space="PSUM") as psK:
                # XT as one [128, group, t] tile so 4 transposes share one
                # PSUM tile and drain with a single wide copy
                xt = p1.tile([PD, NG, T], BF16, name="xt")

                # weight matrices prefetched up front, dispatch split across
                # the scalar and gpsimd DMA rings (sync is busy with x);
                # the late-phase tensors (wvp etc.) queue behind on gpsimd
                wsb = {}
                for wname, w_d in (("k", wk_d), ("q", wq_d)):
                    for ci in range(NG):
                        wt = p1.tile([PD, C], BF16, tag="w", bufs=16,
                                     name=f"w{wname}{ci}")
                        eng = nc.scalar if (wname, ci) < ("k", 4) else nc.gpsimd
                        for cc in range(2):
                            eng.dma_start(
                                out=wt[:, cc * 512:(cc + 1) * 512],
                                in_=w_d[ci * PD:(ci + 1) * PD,
                                        cc * 512:(cc + 1) * 512])
                        wsb[(wname, ci)] = wt
                load_late_tensors()

                # chunk-major: transpose the 4 t-blocks of chunk ch, then
                # run every K/Q chain for that chunk while the next chunk's
                # x tiles / trig stream in
                for ch in range(NSP):
                    trig = {}
                    for e in range(NPR):
                        csl = slice(ch * TCH, (ch + 1) * TCH)
                        cs = p1.tile([PD, TCH], BF16, tag="trig", bufs=10,
                                     name=f"cs{e}_{ch}")
                        nc.sync.dma_start(
                            out=cs, in_=cos_d[e * PD:(e + 1) * PD, csl])
                        sn = p1.tile([PD, TCH], BF16, tag="trig", bufs=10,
                                     name=f"sn{e}_{ch}")
                        nc.sync.dma_start(
                            out=sn, in_=sin_d[e * PD:(e + 1) * PD, csl])
                        trig[e] = (cs, sn)
                    if ch + 1 < NSP:
                        for j in range(4 * (ch + 1), 4 * (ch + 1) + 4):
                            load_xs(j, 2)

                    for j in range(4 * ch, 4 * ch + 4):
                        for half in range(2):
                            g0 = half * 4
                            ps = pstr.tile([PD, 4, PD], BF16, tag="ps_tr",
                                           name=f"ptr{j}_{half}")
                            for m in range(4):
                                g = g0 + m
                                nc.tensor.transpose(
                                    ps[:, m, :],
                                    xs[j][:, g * PD:(g + 1) * PD], ident)
                            dst = xt[:, g0:g0 + 4, j * PD:(j + 1) * PD]
                            if half == 0:
                                nc.scalar.activation(dst, ps, AF.Copy)
                            else:
                                nc.vector.tensor_copy(dst, ps)

                    for wname, brt, dst8 in (("k", bkr, kr8),
                                             ("q", bqr, qr8)):
                        for e in range(NPR):
                            o = e + NPR
                            tmp = {}
                            for g in (e, o):
                                ps = psK.tile([PD, TCH], F32, tag="ps_kq",
                                              name=f"pkq{wname}{g}_{ch}")
                                for ci in range(NG):
                                    nc.tensor.matmul(
                                        ps,
                                        lhsT=wsb[(wname, ci)][:,
                                                              g * PD:(g + 1) * PD],
                                        rhs=xt[:, ci, ch * TCH:(ch + 1) * TCH],
                                        start=(ci == 0), stop=(ci == NG - 1))
                                kt = p1.tile([PD, TCH], BF16, tag="kttmp",
                                             bufs=10, name=f"kt{wname}{g}_{ch}")
                                nc.scalar.activation(kt, ps, AF.Identity,
                                                     bias=brt[:, g:g + 1])
                                tmp[g] = kt
                            cs, sn = trig[e]
                            ze, zo = tmp[e], tmp[o]
                            t1 = p1.tile([PD, TCH], BF16, tag="rot", bufs=6,
                                         name=f"r1{wname}{e}_{ch}")
                            nc.vector.tensor_mul(t1, ze, cs)
                            t2 = p1.tile([PD, TCH], BF16, tag="rot", bufs=6,
                                         name=f"r2{wname}{e}_{ch}")
                            nc.vector.tensor_mul(t2, zo, sn)
                            nc.vector.tensor_add(dst8[e][ch][:, 0, :], t1, t2)
                            t3 = p1.tile([PD, TCH], BF16, tag="rot", bufs=6,
                                         name=f"r3{wname}{e}_{ch}")
                            nc.vector.tensor_mul(t3, zo, cs)
                            t4 = p1.tile([PD, TCH], BF16, tag="rot", bufs=6,
                                         name=f"r4{wname}{e}_{ch}")
                            nc.vector.tensor_mul(t4, ze, sn)
                            nc.vector.tensor_sub(dst8[e][ch][:, 1, :], t3, t4)

            # ---------------- phases 2-4: attention + projection ---------
            with tc.tile_pool(name="at", bufs=1) as at, \
                 tc.tile_pool(name="psS", bufs=3, space="PSUM") as psS, \
                 tc.tile_pool(name="psB", bufs=2, space="PSUM") as psB, \
                 tc.tile_pool(name="psP", bufs=2, space="PSUM") as psP:
                for s in range(NSP):
                    nj = 4 * (s + 1)
                    stact = []
                    for j in range(nj):
                        d = j - 4 * s
                        st = at.tile([PD, TCH], BF16, tag="stact", bufs=20,
                                     name=f"st{s}_{j}")
                        se = at.tile([PD, TCH], F32, tag="stexp", bufs=4,
                                     name=f"se{s}_{j}")
                        ps = psS.tile([PD, TCH], F32, tag="ps_sc",
                                      name=f"pst{s}_{j}")
                        h0 = 1 if d >= 2 else 0   # left half skip (causal)
                        jc, jo = j // 4, j % 4
                        for h in range(h0, 2):
                            for g in range(NPR):
                                nc.tensor.matmul(
                                    ps[:, h * HF:(h + 1) * HF],
                                    lhsT=qr8[g][jc][:, :,
                                                    jo * PD:(jo + 1) * PD],
                                    rhs=kr8[g][s][:, :,
                                                  h * HF:(h + 1) * HF],
                                    start=(g == 0), stop=(g == NPR - 1),
                                    perf_mode=DR)
                        if h0:
                            # fully above the diagonal: never computed
                            nc.gpsimd.memset(st[:, :HF], 0.0)
                        asl = slice(h0 * HF, TCH)
                        # softplus(x) = ln(1 + exp(x)); scores/sqrt(C) are
                        # bounded to a few units so exp cannot overflow
                        nc.scalar.activation(se[:, asl], ps[:, asl], AF.Exp,
                                             scale=INV_SQRT_C)
                        nc.scalar.activation(st[:, asl], se[:, asl],
                                             AF.Ln, bias=1.0)
                        if d >= 0:
                            nc.vector.tensor_mul(st, st, mskt[d])
                        stact.append(st)

                    # masked-score rowsum for the rank-1 bv*Wp term
                    psr = psS.tile([1, TCH], F32, tag="ps_rs", bufs=1,
                                   name=f"prs{s}")
                    for j in range(nj):
                        nc.tensor.matmul(psr, lhsT=ones, rhs=stact[j],
                                         start=(j == 0), stop=(j == nj - 1))
                    nc.scalar.activation(rsum[s], psr, AF.Copy)

                    ot = []
                    for g in range(NG):
                        ps2 = psB.tile([PD, TCH], F32, tag="ps_ot",
                                       name=f"pot{s}_{g}")
                        for j in range(nj):
                            nc.tensor.matmul(
                                ps2,
                                lhsT=xs[j][:, g * PD:(g + 1) * PD],
                                rhs=stact[j],
                                start=(j == 0), stop=(j == nj - 1))
                        o = at.tile([PD, TCH], BF16, tag="ot", bufs=16,
                                    name=f"ot{s}_{g}")
                        if g % 2 == 0:
                            nc.scalar.activation(o, ps2, AF.Copy)
                        else:
                            nc.vector.tensor_copy(o, ps2)
                        ot.append(o)

                    for tt in range(4):
                        trow = s * TCH + tt * PD
                        for h in range(2):
                            ps = psP.tile([PD, TCH], F32, tag="ps_mm",
                                          name=f"ppr{s}_{tt}_{h}")
                            for g in range(NG):
                                nc.tensor.matmul(
                                    ps,
                                    lhsT=ot[g][:, tt * PD:(tt + 1) * PD],
                                    rhs=wpsb[g][:, h * TCH:(h + 1) * TCH],
                                    start=(g == 0), stop=False)
                            # rank-1 rowsum x (bv Wp) joins the same PSUM
                            # accumulation group as a K=1 f32r matmul
                            nc.tensor.matmul(
                                ps,
                                lhsT=rsum[s][0:1, tt * PD:(tt + 1) * PD],
                                rhs=bvp[0:1, h * TCH:(h + 1) * TCH],
                                start=False, stop=True)
                            ob = at.tile([PD, TCH], F32, tag="ob", bufs=4,
                                         name=f"ob{s}_{tt}_{h}")
                            nc.vector.tensor_add(ob, ps,
                                                 bpb[:, h * TCH:(h + 1) * TCH])
                            for cc in range(4):
                                nc.sync.dma_start(
                                    out=out_d[trow:trow + PD,
                                              h * TCH + cc * PD:
                                              h * TCH + (cc + 1) * PD],
                                    in_=ob[:, cc * PD:(cc + 1) * PD])
    nc.finalize()
    return nc


def _static_tables():
    if "tables" in _CACHE:
        return _CACHE["tables"]
    perm = np.concatenate([np.arange(0, C, 2), np.arange(1, C, 2)])
    j = np.arange(H, dtype=np.float64)
    t = (T - 1 - np.arange(T)).astype(np.float64)
    ang = np.outer(j, t)                      # [H, T], angle of pair j at time t
    cosT = np.cos(ang).astype(ml_dtypes.bfloat16)
    sinT = np.sin(ang).astype(ml_dtypes.bfloat16)
    a = np.arange(PD)[:, None]
    b = np.arange(TCH)[None, :]
    masks = np.stack([(a + PD * d <= b) for d in range(NSP)])
    masks = masks.astype(ml_dtypes.bfloat16)
    ident = np.eye(PD, dtype=ml_dtypes.bfloat16)
    _CACHE["tables"] = (perm, cosT, sinT, masks, ident)
    return _CACHE["tables"]


def prepare(x, Wk, bk, Wq, bq, Wv, bv, Wp, bp):
    """Build (cached) the Bass program and the per-core input maps."""
    x = np.asarray(x, dtype=np.float32)
    Wk, bk = np.asarray(Wk, np.float32), np.asarray(bk, np.float32)
    Wq, bq = np.asarray(Wq, np.float32), np.asarray(bq, np.float32)
    Wv, bv = np.asarray(Wv, np.float32), np.asarray(bv, np.float32)
    Wp, bp = np.asarray(Wp, np.float32), np.asarray(bp, np.float32)

    perm, cosT, sinT, masks, ident = _static_tables()

    wk = np.ascontiguousarray(Wk[:, perm]).astype(ml_dtypes.bfloat16)
    wq = np.ascontiguousarray(Wq[:, perm]).astype(ml_dtypes.bfloat16)
    wvp = (Wv.astype(np.float64) @ Wp.astype(np.float64))
    bvp = (bv.astype(np.float64) @ Wp.astype(np.float64))
    wvp = wvp.astype(ml_dtypes.bfloat16)
    bvp = np.ascontiguousarray(bvp.reshape(1, C)).astype(np.float32)
    bkr = np.ascontiguousarray(bk[perm].reshape(NG, PD).T).astype(np.float32)
    bqr = np.ascontiguousarray(bq[perm].reshape(NG, PD).T).astype(np.float32)
    bpb = np.ascontiguousarray(np.broadcast_to(bp, (PD, C))).astype(np.float32)

    if "nc" not in _CACHE:
        _CACHE["nc"] = _build_nc()
    nc = _CACHE["nc"]

    shared = dict(wk=wk, wq=wq, wvp=wvp, bkr=bkr, bqr=bqr,
                  bvp=bvp, bpb=bpb, cosT=cosT, sinT=sinT, masks=masks,
                  ident=ident)
    xb = x.astype(ml_dtypes.bfloat16)
    in_maps = [dict(x=np.ascontiguousarray(xb[i]), **shared)
               for i in range(NCORES)]
    return nc, in_maps


def kernel(x, Wk, bk, Wq, bq, Wv, bv, Wp, bp):
    global LAST_RESULT
    nc, in_maps = prepare(x, Wk, bk, Wq, bq, Wv, bv, Wp, bp)
    res = run_bass_kernel_spmd(nc, in_maps, list(range(NCORES)))
    LAST_RESULT = res
    out = np.stack([res.results[i]["out"] for i in range(NCORES)], axis=0)
    return out.astype(np.float32)


# revision 31
# speedup vs baseline: 1.0127x; 1.0127x over previous
"""Trainium2 Bass kernel for nn_Attention_41996190220419.

Single-head causal attention with softplus weights and a time-flipped
rotary embedding, B=8 T=2048 C=1024 fp32.

Sharding: pure data-parallel over batch (1 batch element per NeuronCore,
8 cores, no collectives).

Optimizations over the 413us baseline (all per-core):
  * Wv/Wp folding: out = (wei @ x) @ (Wv Wp) + rowsum(wei) x (bv Wp) + bp.
    The V GEMM (256 N=512 matmuls) disappears; wei@x consumes x in its
    native [t, c] layout (the same tiles the transposes read), and the
    rank-1 rowsum term is PE-cheap: per span one M=1 accumulation chain
    over the masked score tiles plus one K=1 float32r matmul appended to
    each projection accumulation group.
  * Scores in fp8 e4m3 with MatmulPerfMode.DoubleRow (2 contraction rows
    per PE cell): the rotated K/Q are written by the DVE straight into
    [128, 2, T] paired-group tiles; each score tile is 2 halves x 4
    DoubleRow matmuls (K=256 each) instead of 8 bf16 N=512 matmuls.
    Numerically validated: L2 rel err ~6.6e-3 (budget 2e-2).
  * Causal skip: diagonal-block score halves that are fully above the
    diagonal (d>=2, left half) are never computed (gpsimd memset zeroes
    the st half instead).
  * Rotation arithmetic in bf16 (2x DVE throughput); error is far below
    the fp8 quantization already applied to the rotated K/Q.

Per-core phases (matmuls bf16 with fp32 PSUM accumulate unless noted):
  0: x [T,C] bf16 -> 16 resident xs tiles; XT via PE transposes
  1: KT/QT = W^T XT (+bias via ACT), bf16 rotation on DVE -> fp8 pair
     tiles kr8/qr8 [128, 2, T]
  2: per 512-span: ST[j,i] via fp8 DoubleRow (halves of 256), softplus =
     Ln(Exp(x)+1) on ACT, diagonal masks on DVE, rowsum via M=1 chain
  3: OT[c,i] = sum_j x[j,c] ST[j,i] (PSUM accumulate over j, bf16)
  4: OUT[t,:] = sum_c OT[c,t] Wvp[c,:] + rowsum*bvp (K=1 f32r matmul in
     the same PSUM group) + bp -> DRAM

The even/odd rotation pairs are tile-level structure: Wk/Wq columns (and
bk/bq) are host-permuted to [evens|odds]; scores are invariant to any
channel permutation applied to both K and Q. cos/sin tables, masks and
the identity are host-precomputed inputs.
"""

import os
import sys

if "/opt/trn_rl_repo" not in sys.path:
    sys.path.insert(0, "/opt/trn_rl_repo")

import numpy as np
import ml_dtypes

import concourse.bass as bass
import concourse.bacc as bacc
import concourse.mybir as mybir
import concourse.tile as tile
from concourse.bass_utils import run_bass_kernel_spmd

B, T, C = 8, 2048, 1024
H = C // 2
NCORES = 8
PD = 128
TCH = 512                 # t-chunk width (phase 1) == i-span width (attention)
HF = 256                  # fp8 DoubleRow moving half-width
NT = T // PD              # 16
NSP = T // TCH            # 4
NG = C // PD              # 8
NPR = NG // 2             # 4 fp8 pair tiles
BF16 = mybir.dt.bfloat16
F32 = mybir.dt.float32
F32R = mybir.dt.float32r
F8 = mybir.dt.float8e4
DR = mybir.MatmulPerfMode.DoubleRow
AF = mybir.ActivationFunctionType
INV_SQRT_C = float(C) ** -0.5

_CACHE = {}

LAST_RESULT = None  # BassKernelResults of the most recent run (for profiling)


def _patch_act_tables():
    """Force every ACT func we use (Copy/Identity/Exp/Ln) to resolve to the
    single `natural_log_exp_and_others` table so the Exp/Ln alternation in
    the softplus does not thrash ACT_TABLE_LOADs (1.3us each).
    Table ids are positional, so keep the dict order and only strip
    functions from the other tables."""
    if _CACHE.get("act_patched"):
        return
    from concourse import hw_specs
    orig = hw_specs.get_activation_tables
    combined = "natural_log_exp_and_others"

    def patched(arch):
        tables = orig(arch)
        if combined in tables:
            keep = tables[combined]
            tables = {
                name: (s if name == combined else (s - keep))
                for name, s in tables.items()
            }
        return tables

    hw_specs.get_activation_tables = patched
    bacc.get_activation_tables = patched
    _CACHE["act_patched"] = True


def _build_nc():
    _patch_act_tables()
    nc = bacc.Bacc("TRN2", target_bir_lowering=False, debug=False,
                   num_devices=NCORES)

    x_d = nc.dram_tensor("x", [T, C], BF16, kind="ExternalInput").ap()
    wk_d = nc.dram_tensor("wk", [C, C], BF16, kind="ExternalInput").ap()
    wq_d = nc.dram_tensor("wq", [C, C], BF16, kind="ExternalInput").ap()
    wvp_d = nc.dram_tensor("wvp", [C, C], BF16, kind="ExternalInput").ap()
    bkr_d = nc.dram_tensor("bkr", [PD, NG], F32, kind="ExternalInput").ap()
    bqr_d = nc.dram_tensor("bqr", [PD, NG], F32, kind="ExternalInput").ap()
    bvp_d = nc.dram_tensor("bvp", [1, C], F32R, kind="ExternalInput").ap()
    bpb_d = nc.dram_tensor("bpb", [PD, C], F32, kind="ExternalInput").ap()
    cos_d = nc.dram_tensor("cosT", [H, T], BF16, kind="ExternalInput").ap()
    sin_d = nc.dram_tensor("sinT", [H, T], BF16, kind="ExternalInput").ap()
    msk_d = nc.dram_tensor("masks", [NSP, PD, TCH], BF16,
                           kind="ExternalInput").ap()
    idn_d = nc.dram_tensor("ident", [PD, PD], BF16, kind="ExternalInput").ap()
    out_d = nc.dram_tensor("out", [T, C], F32, kind="ExternalOutput").ap()

    with tile.TileContext(nc) as tc:
        with tc.tile_pool(name="persist", bufs=1) as pp:

            # resident x in native [t, c] layout (transpose source + OT lhsT).
            # DMA queue engines run ~16 GB/s each, so big tiles are split
            # into chunks that land on different queues; the first two x
            # tiles (critical path for the transposes) are split finest.
            xs = [pp.tile([PD, C], BF16, tag=f"xs{j}", name=f"xs{j}")
                  for j in range(NT)]

            def load_xs(j, nchunk, eng=None):
                eng = eng or nc.sync
                w = C // nchunk
                for cc in range(nchunk):
                    eng.dma_start(
                        out=xs[j][:, cc * w:(cc + 1) * w],
                        in_=x_d[j * PD:(j + 1) * PD, cc * w:(cc + 1) * w])

            # first chunk's x tiles split finest and spread across all three
            # DMA rings so the transposes can start ~12us in
            load_xs(0, 4)
            ident = pp.tile([PD, PD], BF16, name="ident")
            nc.sync.dma_start(out=ident, in_=idn_d)
            load_xs(1, 4, nc.scalar)
            load_xs(2, 4, nc.gpsimd)
            load_xs(3, 4)
            # rotated K/Q fp8 pair tiles: slab 0 = even group e, slab 1 = odd
            # group e+4 (DoubleRow contracts both slabs per matmul); split
            # per span so attention span s only depends on chunk-s rotations
            kr8 = [[pp.tile([PD, 2, TCH], F8, tag=f"kr{e}_{s}",
                            name=f"kr{e}_{s}") for s in range(NSP)]
                   for e in range(NPR)]
            qr8 = [[pp.tile([PD, 2, TCH], F8, tag=f"qr{e}_{s}",
                            name=f"qr{e}_{s}") for s in range(NSP)]
                   for e in range(NPR)]

            bkr = pp.tile([PD, NG], F32, name="bkr")
            nc.sync.dma_start(out=bkr, in_=bkr_d)
            bqr = pp.tile([PD, NG], F32, name="bqr")
            nc.sync.dma_start(out=bqr, in_=bqr_d)
            ones = pp.tile([PD, 1], BF16, name="ones")
            nc.gpsimd.memset(ones, 1.0)
            # everything below is needed late; dispatched from gpsimd's DMA
            # ring so the sync ring stays dedicated to x and trig
            mskt = []
            for d in range(NSP):
                m = pp.tile([PD, TCH], BF16, tag=f"msk{d}", name=f"msk{d}")
                mskt.append(m)
            bvp = pp.tile([1, C], F32R, name="bvp")
            bpb = pp.tile([PD, C], F32, name="bpb")
            # masked-score rowsums, one [1, TCH] f32 row per span
            rsum = [pp.tile([1, TCH], F32R, tag=f"rs{s}", name=f"rs{s}")
                    for s in range(NSP)]
            wpsb = [pp.tile([PD, C], BF16, tag=f"wp{ci}", name=f"wp{ci}")
                    for ci in range(NG)]

            def load_late_tensors():
                for ci in range(NG):
                    for cc in range(2):
                        nc.gpsimd.dma_start(
                            out=wpsb[ci][:, cc * 512:(cc + 1) * 512],
                            in_=wvp_d[ci * PD:(ci + 1) * PD,
                                      cc * 512:(cc + 1) * 512])
                for d in range(NSP):
                    nc.gpsimd.dma_start(out=mskt[d], in_=msk_d[d])
                nc.gpsimd.dma_start(out=bvp, in_=bvp_d)
                nc.gpsimd.dma_start(out=bpb, in_=bpb_d)

            # ---------------- phase 0 + 1: XT, rotated K/Q ----------------
            with tc.tile_pool(name="p1", bufs=1) as p1, \
                 tc.tile_pool(name="pstr", bufs=4, space="PSUM") as pstr, \
                 tc.tile_pool(name="psK", bufs=3, space="PSUM") as psK:
                # XT as one [128, group, t] tile so 4 transposes share one
                # PSUM tile and drain with a single wide copy
                xt = p1.tile([PD, NG, T], BF16, name="xt")

                # weight matrices prefetched up front, dispatch split across
                # the scalar and gpsimd DMA rings (sync is busy with x);
                # the late-phase tensors (wvp etc.) queue behind on gpsimd
                wsb = {}
                for wname, w_d in (("k", wk_d), ("q", wq_d)):
                    for ci in range(NG):
                        wt = p1.tile([PD, C], BF16, tag="w", bufs=16,
                                     name=f"w{wname}{ci}")
                        eng = nc.scalar if (wname, ci) < ("k", 4) else nc.gpsimd
                        for cc in range(2):
                            eng.dma_start(
                                out=wt[:, cc * 512:(cc + 1) * 512],
                                in_=w_d[ci * PD:(ci + 1) * PD,
                                        cc * 512:(cc + 1) * 512])
                        wsb[(wname, ci)] = wt
                load_late_tensors()

                # chunk-major: transpose the 4 t-blocks of chunk ch, then
                # run every K/Q chain for that chunk while the next chunk's
                # x tiles / trig stream in
                for ch in range(NSP):
                    trig = {}
                    for e in range(NPR):
                        csl = slice(ch * TCH, (ch + 1) * TCH)
                        cs = p1.tile([PD, TCH], BF16, tag="trig", bufs=10,
                                     name=f"cs{e}_{ch}")
                        nc.sync.dma_start(
                            out=cs, in_=cos_d[e * PD:(e + 1) * PD, csl])
                        sn = p1.tile([PD, TCH], BF16, tag="trig", bufs=10,
                                     name=f"sn{e}_{ch}")
                        nc.sync.dma_start(
                            out=sn, in_=sin_d[e * PD:(e + 1) * PD, csl])
                        trig[e] = (cs, sn)
                    if ch + 1 < NSP:
                        for j in range(4 * (ch + 1), 4 * (ch + 1) + 4):
                            load_xs(j, 2)

                    for j in range(4 * ch, 4 * ch + 4):
                        for half in range(2):
                            g0 = half * 4
                            ps = pstr.tile([PD, 4, PD], BF16, tag="ps_tr",
                                           name=f"ptr{j}_{half}")
                            for m in range(4):
                                g = g0 + m
                                nc.tensor.transpose(
                                    ps[:, m, :],
                                    xs[j][:, g * PD:(g + 1) * PD], ident)
                            dst = xt[:, g0:g0 + 4, j * PD:(j + 1) * PD]
                            if half == 0:
                                nc.scalar.activation(dst, ps, AF.Copy)
                            else:
                                nc.vector.tensor_copy(dst, ps)

                    for wname, brt, dst8 in (("k", bkr, kr8),
                                             ("q", bqr, qr8)):
                        for e in range(NPR):
                            o = e + NPR
                            tmp = {}
                            for g in (e, o):
                                ps = psK.tile([PD, TCH], F32, tag="ps_kq",
                                              name=f"pkq{wname}{g}_{ch}")
                                for ci in range(NG):
                                    nc.tensor.matmul(
                                        ps,
                                        lhsT=wsb[(wname, ci)][:,
                                                              g * PD:(g + 1) * PD],
                                        rhs=xt[:, ci, ch * TCH:(ch + 1) * TCH],
                                        start=(ci == 0), stop=(ci == NG - 1))
                                kt = p1.tile([PD, TCH], BF16, tag="kttmp",
                                             bufs=10, name=f"kt{wname}{g}_{ch}")
                                nc.scalar.activation(kt, ps, AF.Identity,
                                                     bias=brt[:, g:g + 1])
                                tmp[g] = kt
                            cs, sn = trig[e]
                            ze, zo = tmp[e], tmp[o]
                            t1 = p1.tile([PD, TCH], BF16, tag="rot", bufs=6,
                                         name=f"r1{wname}{e}_{ch}")
                            nc.vector.tensor_mul(t1, ze, cs)
                            t2 = p1.tile([PD, TCH], BF16, tag="rot", bufs=6,
                                         name=f"r2{wname}{e}_{ch}")
                            nc.vector.tensor_mul(t2, zo, sn)
                            nc.vector.tensor_add(dst8[e][ch][:, 0, :], t1, t2)
                            t3 = p1.tile([PD, TCH], BF16, tag="rot", bufs=6,
                                         name=f"r3{wname}{e}_{ch}")
                            nc.vector.tensor_mul(t3, zo, cs)
                            t4 = p1.tile([PD, TCH], BF16, tag="rot", bufs=6,
                                         name=f"r4{wname}{e}_{ch}")
                            nc.vector.tensor_mul(t4, ze, sn)
                            nc.vector.tensor_sub(dst8[e][ch][:, 1, :], t3, t4)

            # ---------------- phases 2-4: attention + projection ---------
            with tc.tile_pool(name="at", bufs=1) as at, \
                 tc.tile_pool(name="psS", bufs=3, space="PSUM") as psS, \
                 tc.tile_pool(name="psB", bufs=2, space="PSUM") as psB, \
                 tc.tile_pool(name="psP", bufs=2, space="PSUM") as psP:
                for s in range(NSP):
                    nj = 4 * (s + 1)
                    stact = []
                    for j in range(nj):
                        d = j - 4 * s
                        st = at.tile([PD, TCH], BF16, tag="stact", bufs=20,
                                     name=f"st{s}_{j}")
                        se = at.tile([PD, TCH], F32, tag="stexp", bufs=4,
                                     name=f"se{s}_{j}")
                        ps = psS.tile([PD, TCH], F32, tag="ps_sc",
                                      name=f"pst{s}_{j}")
                        h0 = 1 if d >= 2 else 0   # left half skip (causal)
                        jc, jo = j // 4, j % 4
                        for h in range(h0, 2):
                            for g in range(NPR):
                                nc.tensor.matmul(
                                    ps[:, h * HF:(h + 1) * HF],
                                    lhsT=qr8[g][jc][:, :,
                                                    jo * PD:(jo + 1) * PD],
                                    rhs=kr8[g][s][:, :,
                                                  h * HF:(h + 1) * HF],
                                    start=(g == 0), stop=(g == NPR - 1),
                                    perf_mode=DR)
                        if h0:
                            # fully above the diagonal: never computed
                            nc.gpsimd.memset(st[:, :HF], 0.0)
                        asl = slice(h0 * HF, TCH)
                        # softplus(x) = ln(1 + exp(x)); scores/sqrt(C) are
                        # bounded to a few units so exp cannot overflow
                        nc.scalar.activation(se[:, asl], ps[:, asl], AF.Exp,
                                             scale=INV_SQRT_C)
                        nc.scalar.activation(st[:, asl], se[:, asl],
                                             AF.Ln, bias=1.0)
                        if d >= 0:
                            nc.vector.tensor_mul(st, st, mskt[d])
                        stact.append(st)

                    # masked-score rowsum for the rank-1 bv*Wp term
                    psr = psS.tile([1, TCH], F32, tag="ps_rs", bufs=1,
                                   name=f"prs{s}")
                    for j in range(nj):
                        nc.tensor.matmul(psr, lhsT=ones, rhs=stact[j],
                                         start=(j == 0), stop=(j == nj - 1))
                    nc.scalar.activation(rsum[s], psr, AF.Copy)

                    ot = []
                    for g in range(NG):
                        ps2 = psB.tile([PD, TCH], F32, tag="ps_ot",
                                       name=f"pot{s}_{g}")
                        for j in range(nj):
                            nc.tensor.matmul(
                                ps2,
                                lhsT=xs[j][:, g * PD:(g + 1) * PD],
                                rhs=stact[j],
                                start=(j == 0), stop=(j == nj - 1))
                        o = at.tile([PD, TCH], BF16, tag="ot", bufs=16,
                                    name=f"ot{s}_{g}")
                        if g % 2 == 0:
                            nc.scalar.activation(o, ps2, AF.Copy)
                        else:
                            nc.vector.tensor_copy(o, ps2)
                        ot.append(o)

                    for tt in range(4):
                        trow = s * TCH + tt * PD
                        for h in range(2):
                            ps = psP.tile([PD, TCH], F32, tag="ps_mm",
                                          name=f"ppr{s}_{tt}_{h}")
                            for g in range(NG):
                                nc.tensor.matmul(
                                    ps,
                                    lhsT=ot[g][:, tt * PD:(tt + 1) * PD],
                                    rhs=wpsb[g][:, h * TCH:(h + 1) * TCH],
                                    start=(g == 0), stop=False)
                            # rank-1 rowsum x (bv Wp) joins the same PSUM
                            # accumulation group as a K=1 f32r matmul
                            nc.tensor.matmul(
                                ps,
                                lhsT=rsum[s][0:1, tt * PD:(tt + 1) * PD],
                                rhs=bvp[0:1, h * TCH:(h + 1) * TCH],
                                start=False, stop=True)
                            ob = at.tile([PD, TCH], F32, tag="ob", bufs=4,
                                         name=f"ob{s}_{tt}_{h}")
                            nc.vector.tensor_add(ob, ps,
                                                 bpb[:, h * TCH:(h + 1) * TCH])
                            for cc in range(2):
                                nc.sync.dma_start(
                                    out=out_d[trow:trow + PD,
                                              h * TCH + cc * HF:
                                              h * TCH + (cc + 1) * HF],
                                    in_=ob[:, cc * HF:(cc + 1) * HF])
    nc.finalize()
    return nc


def _static_tables():
    if "tables" in _CACHE:
        return _CACHE["tables"]
    perm = np.concatenate([np.arange(0, C, 2), np.arange(1, C, 2)])
    j = np.arange(H, dtype=np.float64)
    t = (T - 1 - np.arange(T)).astype(np.float64)
    ang = np.outer(j, t)                      # [H, T], angle of pair j at time t
    cosT = np.cos(ang).astype(ml_dtypes.bfloat16)
    sinT = np.sin(ang).astype(ml_dtypes.bfloat16)
    a = np.arange(PD)[:, None]
    b = np.arange(TCH)[None, :]
    masks = np.stack([(a + PD * d <= b) for d in range(NSP)])
    masks = masks.astype(ml_dtypes.bfloat16)
    ident = np.eye(PD, dtype=ml_dtypes.bfloat16)
    _CACHE["tables"] = (perm, cosT, sinT, masks, ident)
    return _CACHE["tables"]


def prepare(x, Wk, bk, Wq, bq, Wv, bv, Wp, bp):
    """Build (cached) the Bass program and the per-core input maps."""
    x = np.asarray(x, dtype=np.float32)
    Wk, bk = np.asarray(Wk, np.float32), np.asarray(bk, np.float32)
    Wq, bq = np.asarray(Wq, np.float32), np.asarray(bq, np.float32)
    Wv, bv = np.asarray(Wv, np.float32), np.asarray(bv, np.float32)
    Wp, bp = np.asarray(Wp, np.float32), np.asarray(bp, np.float32)

    perm, cosT, sinT, masks, ident = _static_tables()

    wk = np.ascontiguousarray(Wk[:, perm]).astype(ml_dtypes.bfloat16)
    wq = np.ascontiguousarray(Wq[:, perm]).astype(ml_dtypes.bfloat16)
    wvp = (Wv.astype(np.float64) @ Wp.astype(np.float64))
    bvp = (bv.astype(np.float64) @ Wp.astype(np.float64))
    wvp = wvp.astype(ml_dtypes.bfloat16)
    bvp = np.ascontiguousarray(bvp.reshape(1, C)).astype(np.float32)
    bkr = np.ascontiguousarray(bk[perm].reshape(NG, PD).T).astype(np.float32)
    bqr = np.ascontiguousarray(bq[perm].reshape(NG, PD).T).astype(np.float32)
    bpb = np.ascontiguousarray(np.broadcast_to(bp, (PD, C))).astype(np.float32)

    if "nc" not in _CACHE:
        _CACHE["nc"] = _build_nc()
    nc = _CACHE["nc"]

    shared = dict(wk=wk, wq=wq, wvp=wvp, bkr=bkr, bqr=bqr,
                  bvp=bvp, bpb=bpb, cosT=cosT, sinT=sinT, masks=masks,
                  ident=ident)
    xb = x.astype(ml_dtypes.bfloat16)
    in_maps = [dict(x=np.ascontiguousarray(xb[i]), **shared)
               for i in range(NCORES)]
    return nc, in_maps


def kernel(x, Wk, bk, Wq, bq, Wv, bv, Wp, bp):
    global LAST_RESULT
    nc, in_maps = prepare(x, Wk, bk, Wq, bq, Wv, bv, Wp, bp)
    res = run_bass_kernel_spmd(nc, in_maps, list(range(NCORES)))
    LAST_RESULT = res
    out = np.stack([res.results[i]["out"] for i in range(NCORES)], axis=0)
    return out.astype(np.float32)


# revision 34
# speedup vs baseline: 1.1089x; 1.0950x over previous
"""Trainium2 Bass kernel for nn_Attention_41996190220419.

Single-head causal attention with softplus weights and a time-flipped
rotary embedding, B=8 T=2048 C=1024 fp32.

Sharding: pure data-parallel over batch (1 batch element per NeuronCore,
8 cores, no collectives).

Optimizations over the 413us baseline (all per-core):
  * Wv/Wp folding: out = (wei @ x) @ (Wv Wp) + rowsum(wei) x (bv Wp) + bp.
    The V GEMM (256 N=512 matmuls) disappears; wei@x consumes x in its
    native [t, c] layout (the same tiles the transposes read), and the
    rank-1 rowsum term is PE-cheap: per span one M=1 accumulation chain
    over the masked score tiles plus one K=1 float32r matmul appended to
    each projection accumulation group.
  * Scores in fp8 e4m3 with MatmulPerfMode.DoubleRow (2 contraction rows
    per PE cell): the rotated K/Q are written by the DVE straight into
    [128, 2, T] paired-group tiles; each score tile is 2 halves x 4
    DoubleRow matmuls (K=256 each) instead of 8 bf16 N=512 matmuls.
    Numerically validated: L2 rel err ~6.6e-3 (budget 2e-2).
  * Causal skip: diagonal-block score halves that are fully above the
    diagonal (d>=2, left half) are never computed (gpsimd memset zeroes
    the st half instead).
  * Rotation arithmetic in bf16 (2x DVE throughput); error is far below
    the fp8 quantization already applied to the rotated K/Q.

Per-core phases (matmuls bf16 with fp32 PSUM accumulate unless noted):
  0: x [T,C] bf16 -> 16 resident xs tiles; XT via PE transposes
  1: KT/QT = W^T XT (+bias via ACT), bf16 rotation on DVE -> fp8 pair
     tiles kr8/qr8 [128, 2, T]
  2: per 512-span: ST[j,i] via fp8 DoubleRow (halves of 256), softplus =
     Ln(Exp(x)+1) on ACT, diagonal masks on DVE, rowsum via M=1 chain
  3: OT[c,i] = sum_j x[j,c] ST[j,i] (PSUM accumulate over j, bf16)
  4: OUT[t,:] = sum_c OT[c,t] Wvp[c,:] + rowsum*bvp (K=1 f32r matmul in
     the same PSUM group) + bp -> DRAM

The even/odd rotation pairs are tile-level structure: Wk/Wq columns (and
bk/bq) are host-permuted to [evens|odds]; scores are invariant to any
channel permutation applied to both K and Q. cos/sin tables, masks and
the identity are host-precomputed inputs.
"""

import os
import sys

if "/opt/trn_rl_repo" not in sys.path:
    sys.path.insert(0, "/opt/trn_rl_repo")

import numpy as np
import ml_dtypes

import concourse.bass as bass
import concourse.bacc as bacc
import concourse.mybir as mybir
import concourse.tile as tile
from concourse.bass_utils import run_bass_kernel_spmd

B, T, C = 8, 2048, 1024
H = C // 2
NCORES = 8
PD = 128
TCH = 512                 # t-chunk width (phase 1) == i-span width (attention)
HF = 256                  # fp8 DoubleRow moving half-width
NT = T // PD              # 16
NSP = T // TCH            # 4
NG = C // PD              # 8
NPR = NG // 2             # 4 fp8 pair tiles
BF16 = mybir.dt.bfloat16
F32 = mybir.dt.float32
F32R = mybir.dt.float32r
F8 = mybir.dt.float8e4
DR = mybir.MatmulPerfMode.DoubleRow
AF = mybir.ActivationFunctionType
INV_SQRT_C = float(C) ** -0.5

_CACHE = {}

LAST_RESULT = None  # BassKernelResults of the most recent run (for profiling)


def _patch_act_tables():
    """Force every ACT func we use (Copy/Identity/Exp/Ln) to resolve to the
    single `natural_log_exp_and_others` table so the Exp/Ln alternation in
    the softplus does not thrash ACT_TABLE_LOADs (1.3us each).
    Table ids are positional, so keep the dict order and only strip
    functions from the other tables."""
    if _CACHE.get("act_patched"):
        return
    from concourse import hw_specs
    orig = hw_specs.get_activation_tables
    combined = "natural_log_exp_and_others"

    def patched(arch):
        tables = orig(arch)
        if combined in tables:
            keep = tables[combined]
            tables = {
                name: (s if name == combined else (s - keep))
                for name, s in tables.items()
            }
        return tables

    hw_specs.get_activation_tables = patched
    bacc.get_activation_tables = patched
    _CACHE["act_patched"] = True


def _build_nc():
    _patch_act_tables()
    nc = bacc.Bacc("TRN2", target_bir_lowering=False, debug=False,
                   num_devices=NCORES)

    x_d = nc.dram_tensor("x", [T, C], BF16, kind="ExternalInput").ap()
    wk_d = nc.dram_tensor("wk", [C, C], BF16, kind="ExternalInput").ap()
    wq_d = nc.dram_tensor("wq", [C, C], BF16, kind="ExternalInput").ap()
    wvp_d = nc.dram_tensor("wvp", [C, C], BF16, kind="ExternalInput").ap()
    bkr_d = nc.dram_tensor("bkr", [PD, NG], F32, kind="ExternalInput").ap()
    bqr_d = nc.dram_tensor("bqr", [PD, NG], F32, kind="ExternalInput").ap()
    bvp_d = nc.dram_tensor("bvp", [1, C], F32R, kind="ExternalInput").ap()
    bpb_d = nc.dram_tensor("bpb", [PD, C], F32, kind="ExternalInput").ap()
    cos_d = nc.dram_tensor("cosT", [H, T], BF16, kind="ExternalInput").ap()
    sin_d = nc.dram_tensor("sinT", [H, T], BF16, kind="ExternalInput").ap()
    msk_d = nc.dram_tensor("masks", [NSP, PD, TCH], BF16,
                           kind="ExternalInput").ap()
    idn_d = nc.dram_tensor("ident", [PD, PD], BF16, kind="ExternalInput").ap()
    out_d = nc.dram_tensor("out", [T, C], F32, kind="ExternalOutput").ap()

    with tile.TileContext(nc) as tc:
        with tc.tile_pool(name="persist", bufs=1) as pp:

            # resident x in native [t, c] layout (transpose source + OT lhsT).
            # DMA queue engines run ~16 GB/s each, so big tiles are split
            # into chunks that land on different queues; the first two x
            # tiles (critical path for the transposes) are split finest.
            xs = [pp.tile([PD, C], BF16, tag=f"xs{j}", name=f"xs{j}")
                  for j in range(NT)]

            def load_xs(j, nchunk, eng=None):
                eng = eng or nc.sync
                w = C // nchunk
                for cc in range(nchunk):
                    eng.dma_start(
                        out=xs[j][:, cc * w:(cc + 1) * w],
                        in_=x_d[j * PD:(j + 1) * PD, cc * w:(cc + 1) * w])

            # first chunk's x tiles split finest and spread across all three
            # DMA rings so the transposes can start ~12us in
            load_xs(0, 4)
            ident = pp.tile([PD, PD], BF16, name="ident")
            nc.sync.dma_start(out=ident, in_=idn_d)
            load_xs(1, 4, nc.scalar)
            load_xs(2, 4, nc.gpsimd)
            load_xs(3, 4)
            # rotated K/Q fp8 pair tiles: slab 0 = even group e, slab 1 = odd
            # group e+4 (DoubleRow contracts both slabs per matmul)
            kr8 = [pp.tile([PD, 2, T], F8, tag=f"kr{e}", name=f"kr{e}")
                   for e in range(NPR)]
            qr8 = [pp.tile([PD, 2, T], F8, tag=f"qr{e}", name=f"qr{e}")
                   for e in range(NPR)]

            bkr = pp.tile([PD, NG], F32, name="bkr")
            nc.sync.dma_start(out=bkr, in_=bkr_d)
            bqr = pp.tile([PD, NG], F32, name="bqr")
            nc.sync.dma_start(out=bqr, in_=bqr_d)
            ones = pp.tile([PD, 1], BF16, name="ones")
            nc.gpsimd.memset(ones, 1.0)
            # everything below is needed late; dispatched from gpsimd's DMA
            # ring so the sync ring stays dedicated to x and trig
            mskt = []
            for d in range(NSP):
                m = pp.tile([PD, TCH], BF16, tag=f"msk{d}", name=f"msk{d}")
                mskt.append(m)
            bvp = pp.tile([1, C], F32R, name="bvp")
            bpb = pp.tile([PD, C], F32, name="bpb")
            # masked-score rowsums, one [1, TCH] f32 row per span
            rsum = [pp.tile([1, TCH], F32R, tag=f"rs{s}", name=f"rs{s}")
                    for s in range(NSP)]
            wpsb = [pp.tile([PD, C], BF16, tag=f"wp{ci}", name=f"wp{ci}")
                    for ci in range(NG)]

            def load_late_tensors():
                for ci in range(NG):
                    for cc in range(2):
                        nc.gpsimd.dma_start(
                            out=wpsb[ci][:, cc * 512:(cc + 1) * 512],
                            in_=wvp_d[ci * PD:(ci + 1) * PD,
                                      cc * 512:(cc + 1) * 512])
                for d in range(NSP):
                    nc.gpsimd.dma_start(out=mskt[d], in_=msk_d[d])
                nc.gpsimd.dma_start(out=bvp, in_=bvp_d)
                nc.gpsimd.dma_start(out=bpb, in_=bpb_d)

            # ---------------- phase 0 + 1: XT, rotated K/Q ----------------
            with tc.tile_pool(name="p1", bufs=1) as p1, \
                 tc.tile_pool(name="pstr", bufs=4, space="PSUM") as pstr, \
                 tc.tile_pool(name="psK", bufs=3, space="PSUM") as psK:
                # XT as one [128, group, t] tile so 4 transposes share one
                # PSUM tile and drain with a single wide copy
                xt = p1.tile([PD, NG, T], BF16, name="xt")

                # weight matrices prefetched up front, dispatch split across
                # the scalar and gpsimd DMA rings (sync is busy with x);
                # the late-phase tensors (wvp etc.) queue behind on gpsimd
                wsb = {}
                for wname, w_d in (("k", wk_d), ("q", wq_d)):
                    for ci in range(NG):
                        wt = p1.tile([PD, C], BF16, tag="w", bufs=16,
                                     name=f"w{wname}{ci}")
                        eng = nc.scalar if (wname, ci) < ("k", 4) else nc.gpsimd
                        for cc in range(2):
                            eng.dma_start(
                                out=wt[:, cc * 512:(cc + 1) * 512],
                                in_=w_d[ci * PD:(ci + 1) * PD,
                                        cc * 512:(cc + 1) * 512])
                        wsb[(wname, ci)] = wt
                load_late_tensors()

                # chunk-major: transpose the 4 t-blocks of chunk ch, then
                # run every K/Q chain for that chunk while the next chunk's
                # x tiles / trig stream in
                for ch in range(NSP):
                    trig = {}
                    for e in range(NPR):
                        csl = slice(ch * TCH, (ch + 1) * TCH)
                        cs = p1.tile([PD, TCH], BF16, tag="trig", bufs=10,
                                     name=f"cs{e}_{ch}")
                        nc.sync.dma_start(
                            out=cs, in_=cos_d[e * PD:(e + 1) * PD, csl])
                        sn = p1.tile([PD, TCH], BF16, tag="trig", bufs=10,
                                     name=f"sn{e}_{ch}")
                        nc.sync.dma_start(
                            out=sn, in_=sin_d[e * PD:(e + 1) * PD, csl])
                        trig[e] = (cs, sn)
                    if ch + 1 < NSP:
                        for j in range(4 * (ch + 1), 4 * (ch + 1) + 4):
                            load_xs(j, 2)

                    for j in range(4 * ch, 4 * ch + 4):
                        for half in range(2):
                            g0 = half * 4
                            ps = pstr.tile([PD, 4, PD], BF16, tag="ps_tr",
                                           name=f"ptr{j}_{half}")
                            for m in range(4):
                                g = g0 + m
                                nc.tensor.transpose(
                                    ps[:, m, :],
                                    xs[j][:, g * PD:(g + 1) * PD], ident)
                            dst = xt[:, g0:g0 + 4, j * PD:(j + 1) * PD]
                            if half == 0:
                                nc.scalar.activation(dst, ps, AF.Copy)
                            else:
                                nc.vector.tensor_copy(dst, ps)

                    for wname, brt, dst8 in (("k", bkr, kr8),
                                             ("q", bqr, qr8)):
                        for e in range(NPR):
                            o = e + NPR
                            tmp = {}
                            for g in (e, o):
                                ps = psK.tile([PD, TCH], F32, tag="ps_kq",
                                              name=f"pkq{wname}{g}_{ch}")
                                for ci in range(NG):
                                    nc.tensor.matmul(
                                        ps,
                                        lhsT=wsb[(wname, ci)][:,
                                                              g * PD:(g + 1) * PD],
                                        rhs=xt[:, ci, ch * TCH:(ch + 1) * TCH],
                                        start=(ci == 0), stop=(ci == NG - 1))
                                kt = p1.tile([PD, TCH], BF16, tag="kttmp",
                                             bufs=10, name=f"kt{wname}{g}_{ch}")
                                nc.scalar.activation(kt, ps, AF.Identity,
                                                     bias=brt[:, g:g + 1])
                                tmp[g] = kt
                            sl = slice(ch * TCH, (ch + 1) * TCH)
                            cs, sn = trig[e]
                            ze, zo = tmp[e], tmp[o]
                            t1 = p1.tile([PD, TCH], BF16, tag="rot", bufs=6,
                                         name=f"r1{wname}{e}_{ch}")
                            nc.vector.tensor_mul(t1, ze, cs)
                            t2 = p1.tile([PD, TCH], BF16, tag="rot", bufs=6,
                                         name=f"r2{wname}{e}_{ch}")
                            nc.vector.tensor_mul(t2, zo, sn)
                            nc.vector.tensor_add(dst8[e][:, 0, sl], t1, t2)
                            t3 = p1.tile([PD, TCH], BF16, tag="rot", bufs=6,
                                         name=f"r3{wname}{e}_{ch}")
                            nc.vector.tensor_mul(t3, zo, cs)
                            t4 = p1.tile([PD, TCH], BF16, tag="rot", bufs=6,
                                         name=f"r4{wname}{e}_{ch}")
                            nc.vector.tensor_mul(t4, ze, sn)
                            nc.vector.tensor_sub(dst8[e][:, 1, sl], t3, t4)

            # ---------------- phases 2-4: attention + projection ---------
            with tc.tile_pool(name="at", bufs=1) as at, \
                 tc.tile_pool(name="psS", bufs=3, space="PSUM") as psS, \
                 tc.tile_pool(name="psB", bufs=2, space="PSUM") as psB, \
                 tc.tile_pool(name="psP", bufs=2, space="PSUM") as psP:
                for s in range(NSP):
                    nj = 4 * (s + 1)
                    stact = []
                    for j in range(nj):
                        d = j - 4 * s
                        st = at.tile([PD, TCH], BF16, tag="stact", bufs=20,
                                     name=f"st{s}_{j}")
                        se = at.tile([PD, TCH], F32, tag="stexp", bufs=4,
                                     name=f"se{s}_{j}")
                        ps = psS.tile([PD, TCH], F32, tag="ps_sc",
                                      name=f"pst{s}_{j}")
                        h0 = 1 if d >= 2 else 0   # left half skip (causal)
                        for h in range(h0, 2):
                            for g in range(NPR):
                                nc.tensor.matmul(
                                    ps[:, h * HF:(h + 1) * HF],
                                    lhsT=qr8[g][:, :, j * PD:(j + 1) * PD],
                                    rhs=kr8[g][:, :,
                                               s * TCH + h * HF:
                                               s * TCH + (h + 1) * HF],
                                    start=(g == 0), stop=(g == NPR - 1),
                                    perf_mode=DR)
                        if h0:
                            # fully above the diagonal: never computed
                            nc.gpsimd.memset(st[:, :HF], 0.0)
                        asl = slice(h0 * HF, TCH)
                        # softplus(x) = ln(1 + exp(x)); scores/sqrt(C) are
                        # bounded to a few units so exp cannot overflow
                        nc.scalar.activation(se[:, asl], ps[:, asl], AF.Exp,
                                             scale=INV_SQRT_C)
                        nc.scalar.activation(st[:, asl], se[:, asl],
                                             AF.Ln, bias=1.0)
                        if d >= 0:
                            nc.vector.tensor_mul(st, st, mskt[d])
                        stact.append(st)

                    # masked-score rowsum for the rank-1 bv*Wp term
                    psr = psS.tile([1, TCH], F32, tag="ps_rs", bufs=1,
                                   name=f"prs{s}")
                    for j in range(nj):
                        nc.tensor.matmul(psr, lhsT=ones, rhs=stact[j],
                                         start=(j == 0), stop=(j == nj - 1))
                    nc.scalar.activation(rsum[s], psr, AF.Copy)

                    ot = []
                    for g in range(NG):
                        ps2 = psB.tile([PD, TCH], F32, tag="ps_ot",
                                       name=f"pot{s}_{g}")
                        for j in range(nj):
                            nc.tensor.matmul(
                                ps2,
                                lhsT=xs[j][:, g * PD:(g + 1) * PD],
                                rhs=stact[j],
                                start=(j == 0), stop=(j == nj - 1))
                        o = at.tile([PD, TCH], BF16, tag="ot", bufs=16,
                                    name=f"ot{s}_{g}")
                        if g % 2 == 0:
                            nc.scalar.activation(o, ps2, AF.Copy)
                        else:
                            nc.vector.tensor_copy(o, ps2)
                        ot.append(o)

                    for tt in range(4):
                        trow = s * TCH + tt * PD
                        for h in range(2):
                            ps = psP.tile([PD, TCH], F32, tag="ps_mm",
                                          name=f"ppr{s}_{tt}_{h}")
                            for g in range(NG):
                                nc.tensor.matmul(
                                    ps,
                                    lhsT=ot[g][:, tt * PD:(tt + 1) * PD],
                                    rhs=wpsb[g][:, h * TCH:(h + 1) * TCH],
                                    start=(g == 0), stop=False)
                            # rank-1 rowsum x (bv Wp) joins the same PSUM
                            # accumulation group as a K=1 f32r matmul
                            nc.tensor.matmul(
                                ps,
                                lhsT=rsum[s][0:1, tt * PD:(tt + 1) * PD],
                                rhs=bvp[0:1, h * TCH:(h + 1) * TCH],
                                start=False, stop=True)
                            ob = at.tile([PD, TCH], F32, tag="ob", bufs=4,
                                         name=f"ob{s}_{tt}_{h}")
                            nc.vector.tensor_add(ob, ps,
                                                 bpb[:, h * TCH:(h + 1) * TCH])
                            for cc in range(2):
                                nc.sync.dma_start(
                                    out=out_d[trow:trow + PD,
                                              h * TCH + cc * HF:
                                              h * TCH + (cc + 1) * HF],
                                    in_=ob[:, cc * HF:(cc + 1) * HF])
    nc.finalize()
    return nc


def _static_tables():
    if "tables" in _CACHE:
        return _CACHE["tables"]
    perm = np.concatenate([np.arange(0, C, 2), np.arange(1, C, 2)])
    j = np.arange(H, dtype=np.float64)
    t = (T - 1 - np.arange(T)).astype(np.float64)
    ang = np.outer(j, t)                      # [H, T], angle of pair j at time t
    cosT = np.cos(ang).astype(ml_dtypes.bfloat16)
    sinT = np.sin(ang).astype(ml_dtypes.bfloat16)
    a = np.arange(PD)[:, None]
    b = np.arange(TCH)[None, :]
    masks = np.stack([(a + PD * d <= b) for d in range(NSP)])
    masks = masks.astype(ml_dtypes.bfloat16)
    ident = np.eye(PD, dtype=ml_dtypes.bfloat16)
    _CACHE["tables"] = (perm, cosT, sinT, masks, ident)
    return _CACHE["tables"]


def prepare(x, Wk, bk, Wq, bq, Wv, bv, Wp, bp):
    """Build (cached) the Bass program and the per-core input maps."""
    x = np.asarray(x, dtype=np.float32)
    Wk, bk = np.asarray(Wk, np.float32), np.asarray(bk, np.float32)
    Wq, bq = np.asarray(Wq, np.float32), np.asarray(bq, np.float32)
    Wv, bv = np.asarray(Wv, np.float32), np.asarray(bv, np.float32)
    Wp, bp = np.asarray(Wp, np.float32), np.asarray(bp, np.float32)

    perm, cosT, sinT, masks, ident = _static_tables()

    wk = np.ascontiguousarray(Wk[:, perm]).astype(ml_dtypes.bfloat16)
    wq = np.ascontiguousarray(Wq[:, perm]).astype(ml_dtypes.bfloat16)
    wvp = (Wv.astype(np.float64) @ Wp.astype(np.float64))
    bvp = (bv.astype(np.float64) @ Wp.astype(np.float64))
    wvp = wvp.astype(ml_dtypes.bfloat16)
    bvp = np.ascontiguousarray(bvp.reshape(1, C)).astype(np.float32)
    bkr = np.ascontiguousarray(bk[perm].reshape(NG, PD).T).astype(np.float32)
    bqr = np.ascontiguousarray(bq[perm].reshape(NG, PD).T).astype(np.float32)
    bpb = np.ascontiguousarray(np.broadcast_to(bp, (PD, C))).astype(np.float32)

    if "nc" not in _CACHE:
        _CACHE["nc"] = _build_nc()
    nc = _CACHE["nc"]

    shared = dict(wk=wk, wq=wq, wvp=wvp, bkr=bkr, bqr=bqr,
                  bvp=bvp, bpb=bpb, cosT=cosT, sinT=sinT, masks=masks,
                  ident=ident)
    xb = x.astype(ml_dtypes.bfloat16)
    in_maps = [dict(x=np.ascontiguousarray(xb[i]), **shared)
               for i in range(NCORES)]
    return nc, in_maps


def kernel(x, Wk, bk, Wq, bq, Wv, bv, Wp, bp):
    global LAST_RESULT
    nc, in_maps = prepare(x, Wk, bk, Wq, bq, Wv, bv, Wp, bp)
    res = run_bass_kernel_spmd(nc, in_maps, list(range(NCORES)))
    LAST_RESULT = res
    out = np.stack([res.results[i]["out"] for i in range(NCORES)], axis=0)
    return out.astype(np.float32)


# revision 38
# speedup vs baseline: 1.2061x; 1.0876x over previous
"""Trainium2 Bass kernel for nn_Attention_41996190220419.

Single-head causal attention with softplus weights and a time-flipped
rotary embedding, B=8 T=2048 C=1024 fp32.

Sharding: pure data-parallel over batch (1 batch element per NeuronCore,
8 cores, no collectives).

Optimizations over the 413us baseline (all per-core):
  * Wv/Wp folding: out = (wei @ x) @ (Wv Wp) + rowsum(wei) x (bv Wp) + bp.
    The V GEMM (256 N=512 matmuls) disappears; wei@x consumes x in its
    native [t, c] layout (the same tiles the transposes read), and the
    rank-1 rowsum term is PE-cheap: per span one M=1 accumulation chain
    over the masked score tiles plus one K=1 float32r matmul appended to
    each projection accumulation group.
  * Scores in fp8 e4m3 with MatmulPerfMode.DoubleRow (2 contraction rows
    per PE cell): the rotated K/Q are written by the DVE straight into
    [128, 2, T] paired-group tiles; each score tile is 2 halves x 4
    DoubleRow matmuls (K=256 each) instead of 8 bf16 N=512 matmuls.
    Numerically validated: L2 rel err ~6.6e-3 (budget 2e-2).
  * Causal skip: diagonal-block score halves that are fully above the
    diagonal (d>=2, left half) are never computed (gpsimd memset zeroes
    the st half instead).
  * Rotation arithmetic in bf16 (2x DVE throughput); error is far below
    the fp8 quantization already applied to the rotated K/Q.

Per-core phases (matmuls bf16 with fp32 PSUM accumulate unless noted):
  0: x [T,C] bf16 -> 16 resident xs tiles; XT via PE transposes
  1: KT/QT = W^T XT (+bias via ACT), bf16 rotation on DVE -> fp8 pair
     tiles kr8/qr8 [128, 2, T]
  2: per 512-span: ST[j,i] via fp8 DoubleRow (halves of 256), softplus =
     Ln(Exp(x)+1) on ACT, diagonal masks on DVE, rowsum via M=1 chain
  3: OT[c,i] = sum_j x[j,c] ST[j,i] (PSUM accumulate over j, bf16)
  4: OUT[t,:] = sum_c OT[c,t] Wvp[c,:] + rowsum*bvp (K=1 f32r matmul in
     the same PSUM group) + bp -> DRAM

The even/odd rotation pairs are tile-level structure: Wk/Wq columns (and
bk/bq) are host-permuted to [evens|odds]; scores are invariant to any
channel permutation applied to both K and Q. cos/sin tables, masks and
the identity are host-precomputed inputs.
"""

import os
import sys

if "/opt/trn_rl_repo" not in sys.path:
    sys.path.insert(0, "/opt/trn_rl_repo")

import numpy as np
import ml_dtypes

import concourse.bass as bass
import concourse.bacc as bacc
import concourse.mybir as mybir
import concourse.tile as tile
from concourse.bass_utils import run_bass_kernel_spmd

B, T, C = 8, 2048, 1024
H = C // 2
NCORES = 8
PD = 128
TCH = 512                 # t-chunk width (phase 1) == i-span width (attention)
HF = 256                  # fp8 DoubleRow moving half-width
NT = T // PD              # 16
NSP = T // TCH            # 4
NG = C // PD              # 8
NPR = NG // 2             # 4 fp8 pair tiles
BF16 = mybir.dt.bfloat16
F32 = mybir.dt.float32
F32R = mybir.dt.float32r
F8 = mybir.dt.float8e4
DR = mybir.MatmulPerfMode.DoubleRow
AF = mybir.ActivationFunctionType
INV_SQRT_C = float(C) ** -0.5

_CACHE = {}

LAST_RESULT = None  # BassKernelResults of the most recent run (for profiling)


def _patch_act_tables():
    """Force every ACT func we use (Copy/Identity/Exp/Ln) to resolve to the
    single `natural_log_exp_and_others` table so the Exp/Ln alternation in
    the softplus does not thrash ACT_TABLE_LOADs (1.3us each).
    Table ids are positional, so keep the dict order and only strip
    functions from the other tables."""
    if _CACHE.get("act_patched"):
        return
    from concourse import hw_specs
    orig = hw_specs.get_activation_tables
    combined = "natural_log_exp_and_others"

    def patched(arch):
        tables = orig(arch)
        if combined in tables:
            keep = tables[combined]
            tables = {
                name: (s if name == combined else (s - keep))
                for name, s in tables.items()
            }
        return tables

    hw_specs.get_activation_tables = patched
    bacc.get_activation_tables = patched
    _CACHE["act_patched"] = True


def _build_nc():
    _patch_act_tables()
    nc = bacc.Bacc("TRN2", target_bir_lowering=False, debug=False,
                   num_devices=NCORES)

    x_d = nc.dram_tensor("x", [T, C], BF16, kind="ExternalInput").ap()
    wk_d = nc.dram_tensor("wk", [C, C], BF16, kind="ExternalInput").ap()
    wq_d = nc.dram_tensor("wq", [C, C], BF16, kind="ExternalInput").ap()
    wvp_d = nc.dram_tensor("wvp", [C, C], BF16, kind="ExternalInput").ap()
    bkr_d = nc.dram_tensor("bkr", [PD, NG], F32, kind="ExternalInput").ap()
    bqr_d = nc.dram_tensor("bqr", [PD, NG], F32, kind="ExternalInput").ap()
    bvp_d = nc.dram_tensor("bvp", [1, C], F32R, kind="ExternalInput").ap()
    bpb_d = nc.dram_tensor("bpb", [PD, C], F32, kind="ExternalInput").ap()
    cos_d = nc.dram_tensor("cosT", [H, T], BF16, kind="ExternalInput").ap()
    sin_d = nc.dram_tensor("sinT", [H, T], BF16, kind="ExternalInput").ap()
    msk_d = nc.dram_tensor("masks", [NSP, PD, TCH], BF16,
                           kind="ExternalInput").ap()
    idn_d = nc.dram_tensor("ident", [PD, PD], BF16, kind="ExternalInput").ap()
    out_d = nc.dram_tensor("out", [T, C], F32, kind="ExternalOutput").ap()

    with tile.TileContext(nc) as tc:
        with tc.tile_pool(name="persist", bufs=1) as pp:

            # resident x in native [t, c] layout (transpose source + OT lhsT).
            # DMA queue engines run ~16 GB/s each, so big tiles are split
            # into chunks that land on different queues; the first two x
            # tiles (critical path for the transposes) are split finest.
            xs = [pp.tile([PD, C], BF16, tag=f"xs{j}", name=f"xs{j}")
                  for j in range(NT)]

            def load_xs(j, nchunk, eng=None):
                eng = eng or nc.sync
                w = C // nchunk
                for cc in range(nchunk):
                    eng.dma_start(
                        out=xs[j][:, cc * w:(cc + 1) * w],
                        in_=x_d[j * PD:(j + 1) * PD, cc * w:(cc + 1) * w])

            # first chunk's x tiles split finest and spread across all three
            # DMA rings so the transposes can start ~12us in
            load_xs(0, 4)
            ident = pp.tile([PD, PD], BF16, name="ident")
            nc.sync.dma_start(out=ident, in_=idn_d)
            load_xs(1, 4, nc.scalar)
            load_xs(2, 4, nc.gpsimd)
            load_xs(3, 4)
            # rotated K/Q fp8 pair tiles: slab 0 = even group e, slab 1 = odd
            # group e+4 (DoubleRow contracts both slabs per matmul)
            kr8 = [pp.tile([PD, 2, T], F8, tag=f"kr{e}", name=f"kr{e}")
                   for e in range(NPR)]
            qr8 = [pp.tile([PD, 2, T], F8, tag=f"qr{e}", name=f"qr{e}")
                   for e in range(NPR)]

            bkr = pp.tile([PD, NG], F32, name="bkr")
            nc.sync.dma_start(out=bkr, in_=bkr_d)
            bqr = pp.tile([PD, NG], F32, name="bqr")
            nc.sync.dma_start(out=bqr, in_=bqr_d)
            ones = pp.tile([PD, 1], BF16, name="ones")
            nc.gpsimd.memset(ones, 1.0)
            # everything below is needed late; dispatched from gpsimd's DMA
            # ring so the sync ring stays dedicated to x and trig
            mskt = []
            for d in range(NSP):
                m = pp.tile([PD, TCH], BF16, tag=f"msk{d}", name=f"msk{d}")
                mskt.append(m)
            bvp = pp.tile([1, C], F32R, name="bvp")
            bpb = pp.tile([PD, C], F32, name="bpb")
            # masked-score rowsums, one [1, TCH] f32 row per span
            rsum = [pp.tile([1, TCH], F32R, tag=f"rs{s}", name=f"rs{s}")
                    for s in range(NSP)]
            wpsb = [pp.tile([PD, C], BF16, tag=f"wp{ci}", name=f"wp{ci}")
                    for ci in range(NG)]

            def load_late_tensors():
                for ci in range(NG):
                    for cc in range(2):
                        nc.gpsimd.dma_start(
                            out=wpsb[ci][:, cc * 512:(cc + 1) * 512],
                            in_=wvp_d[ci * PD:(ci + 1) * PD,
                                      cc * 512:(cc + 1) * 512])
                for d in range(NSP):
                    nc.gpsimd.dma_start(out=mskt[d], in_=msk_d[d])
                nc.gpsimd.dma_start(out=bvp, in_=bvp_d)
                nc.gpsimd.dma_start(out=bpb, in_=bpb_d)

            # ---------------- phase 0 + 1: XT, rotated K/Q ----------------
            with tc.tile_pool(name="p1", bufs=1) as p1, \
                 tc.tile_pool(name="pstr", bufs=4, space="PSUM") as pstr, \
                 tc.tile_pool(name="psK", bufs=4, space="PSUM") as psK:
                # XT as one [128, group, t] tile so 4 transposes share one
                # PSUM tile and drain with a single wide copy
                xt = p1.tile([PD, NG, T], BF16, name="xt")

                # weight matrices prefetched up front, dispatch split across
                # the scalar and gpsimd DMA rings (sync is busy with x);
                # the late-phase tensors (wvp etc.) queue behind on gpsimd
                wsb = {}
                for wname, w_d in (("k", wk_d), ("q", wq_d)):
                    for ci in range(NG):
                        wt = p1.tile([PD, C], BF16, tag="w", bufs=16,
                                     name=f"w{wname}{ci}")
                        eng = nc.scalar if (wname, ci) < ("k", 4) else nc.gpsimd
                        for cc in range(2):
                            eng.dma_start(
                                out=wt[:, cc * 512:(cc + 1) * 512],
                                in_=w_d[ci * PD:(ci + 1) * PD,
                                        cc * 512:(cc + 1) * 512])
                        wsb[(wname, ci)] = wt
                load_late_tensors()

                # chunk-major: transpose the 4 t-blocks of chunk ch, then
                # run every K/Q chain for that chunk while the next chunk's
                # x tiles / trig stream in
                for ch in range(NSP):
                    trig = {}
                    for e in range(NPR):
                        csl = slice(ch * TCH, (ch + 1) * TCH)
                        cs = p1.tile([PD, TCH], BF16, tag="trig", bufs=10,
                                     name=f"cs{e}_{ch}")
                        nc.sync.dma_start(
                            out=cs, in_=cos_d[e * PD:(e + 1) * PD, csl])
                        sn = p1.tile([PD, TCH], BF16, tag="trig", bufs=10,
                                     name=f"sn{e}_{ch}")
                        nc.sync.dma_start(
                            out=sn, in_=sin_d[e * PD:(e + 1) * PD, csl])
                        trig[e] = (cs, sn)
                    if ch + 1 < NSP:
                        for j in range(4 * (ch + 1), 4 * (ch + 1) + 4):
                            load_xs(j, 2)

                    for j in range(4 * ch, 4 * ch + 4):
                        for half in range(2):
                            g0 = half * 4
                            ps = pstr.tile([PD, 4, PD], BF16, tag="ps_tr",
                                           name=f"ptr{j}_{half}")
                            for m in range(4):
                                g = g0 + m
                                nc.tensor.transpose(
                                    ps[:, m, :],
                                    xs[j][:, g * PD:(g + 1) * PD], ident)
                            dst = xt[:, g0:g0 + 4, j * PD:(j + 1) * PD]
                            if half == 0:
                                nc.scalar.activation(dst, ps, AF.Copy)
                            else:
                                nc.vector.tensor_copy(dst, ps)

                    for wname, brt, dst8 in (("k", bkr, kr8),
                                             ("q", bqr, qr8)):
                        for e in range(NPR):
                            o = e + NPR
                            tmp = {}
                            for g in (e, o):
                                ps = psK.tile([PD, TCH], F32, tag="ps_kq",
                                              name=f"pkq{wname}{g}_{ch}")
                                for ci in range(NG):
                                    nc.tensor.matmul(
                                        ps,
                                        lhsT=wsb[(wname, ci)][:,
                                                              g * PD:(g + 1) * PD],
                                        rhs=xt[:, ci, ch * TCH:(ch + 1) * TCH],
                                        start=(ci == 0), stop=(ci == NG - 1))
                                kt = p1.tile([PD, TCH], BF16, tag="kttmp",
                                             bufs=12, name=f"kt{wname}{g}_{ch}")
                                nc.scalar.activation(kt, ps, AF.Identity,
                                                     bias=brt[:, g:g + 1])
                                tmp[g] = kt
                            sl = slice(ch * TCH, (ch + 1) * TCH)
                            cs, sn = trig[e]
                            ze, zo = tmp[e], tmp[o]
                            t1 = p1.tile([PD, TCH], BF16, tag="rot", bufs=8,
                                         name=f"r1{wname}{e}_{ch}")
                            nc.vector.tensor_mul(t1, ze, cs)
                            t2 = p1.tile([PD, TCH], BF16, tag="rot", bufs=8,
                                         name=f"r2{wname}{e}_{ch}")
                            nc.vector.tensor_mul(t2, zo, sn)
                            nc.vector.tensor_add(dst8[e][:, 0, sl], t1, t2)
                            t3 = p1.tile([PD, TCH], BF16, tag="rot", bufs=8,
                                         name=f"r3{wname}{e}_{ch}")
                            nc.vector.tensor_mul(t3, zo, cs)
                            t4 = p1.tile([PD, TCH], BF16, tag="rot", bufs=8,
                                         name=f"r4{wname}{e}_{ch}")
                            nc.vector.tensor_mul(t4, ze, sn)
                            nc.vector.tensor_sub(dst8[e][:, 1, sl], t3, t4)

            # ---------------- phases 2-4: attention + projection ---------
            with tc.tile_pool(name="at", bufs=1) as at, \
                 tc.tile_pool(name="psS", bufs=3, space="PSUM") as psS, \
                 tc.tile_pool(name="psB", bufs=2, space="PSUM") as psB, \
                 tc.tile_pool(name="psP", bufs=2, space="PSUM") as psP:
                for s in range(NSP):
                    nj = 4 * (s + 1)
                    stact = []
                    for j in range(nj):
                        d = j - 4 * s
                        st = at.tile([PD, TCH], BF16, tag="stact", bufs=20,
                                     name=f"st{s}_{j}")
                        se = at.tile([PD, TCH], F32, tag="stexp", bufs=4,
                                     name=f"se{s}_{j}")
                        ps = psS.tile([PD, TCH], F32, tag="ps_sc",
                                      name=f"pst{s}_{j}")
                        h0 = 1 if d >= 2 else 0   # left half skip (causal)
                        for h in range(h0, 2):
                            for g in range(NPR):
                                nc.tensor.matmul(
                                    ps[:, h * HF:(h + 1) * HF],
                                    lhsT=qr8[g][:, :, j * PD:(j + 1) * PD],
                                    rhs=kr8[g][:, :,
                                               s * TCH + h * HF:
                                               s * TCH + (h + 1) * HF],
                                    start=(g == 0), stop=(g == NPR - 1),
                                    perf_mode=DR)
                        if h0:
                            # fully above the diagonal: never computed
                            nc.gpsimd.memset(st[:, :HF], 0.0)
                        asl = slice(h0 * HF, TCH)
                        # softplus(x) = ln(1 + exp(x)); scores/sqrt(C) are
                        # bounded to a few units so exp cannot overflow
                        nc.scalar.activation(se[:, asl], ps[:, asl], AF.Exp,
                                             scale=INV_SQRT_C)
                        nc.scalar.activation(st[:, asl], se[:, asl],
                                             AF.Ln, bias=1.0)
                        if d >= 0:
                            nc.vector.tensor_mul(st, st, mskt[d])
                        stact.append(st)

                    # masked-score rowsum for the rank-1 bv*Wp term
                    psr = psS.tile([1, TCH], F32, tag="ps_rs", bufs=1,
                                   name=f"prs{s}")
                    for j in range(nj):
                        nc.tensor.matmul(psr, lhsT=ones, rhs=stact[j],
                                         start=(j == 0), stop=(j == nj - 1))
                    nc.scalar.activation(rsum[s], psr, AF.Copy)

                    ot = []
                    for g in range(NG):
                        ps2 = psB.tile([PD, TCH], F32, tag="ps_ot",
                                       name=f"pot{s}_{g}")
                        for j in range(nj):
                            nc.tensor.matmul(
                                ps2,
                                lhsT=xs[j][:, g * PD:(g + 1) * PD],
                                rhs=stact[j],
                                start=(j == 0), stop=(j == nj - 1))
                        o = at.tile([PD, TCH], BF16, tag="ot", bufs=16,
                                    name=f"ot{s}_{g}")
                        if g % 2 == 0:
                            nc.scalar.activation(o, ps2, AF.Copy)
                        else:
                            nc.vector.tensor_copy(o, ps2)
                        ot.append(o)

                    for tt in range(4):
                        trow = s * TCH + tt * PD
                        for h in range(2):
                            ps = psP.tile([PD, TCH], F32, tag="ps_mm",
                                          name=f"ppr{s}_{tt}_{h}")
                            for g in range(NG):
                                nc.tensor.matmul(
                                    ps,
                                    lhsT=ot[g][:, tt * PD:(tt + 1) * PD],
                                    rhs=wpsb[g][:, h * TCH:(h + 1) * TCH],
                                    start=(g == 0), stop=False)
                            # rank-1 rowsum x (bv Wp) joins the same PSUM
                            # accumulation group as a K=1 f32r matmul
                            nc.tensor.matmul(
                                ps,
                                lhsT=rsum[s][0:1, tt * PD:(tt + 1) * PD],
                                rhs=bvp[0:1, h * TCH:(h + 1) * TCH],
                                start=False, stop=True)
                            ob = at.tile([PD, TCH], F32, tag="ob", bufs=4,
                                         name=f"ob{s}_{tt}_{h}")
                            nc.vector.tensor_add(ob, ps,
                                                 bpb[:, h * TCH:(h + 1) * TCH])
                            for cc in range(2):
                                nc.sync.dma_start(
                                    out=out_d[trow:trow + PD,
                                              h * TCH + cc * HF:
                                              h * TCH + (cc + 1) * HF],
                                    in_=ob[:, cc * HF:(cc + 1) * HF])
    nc.finalize()
    return nc


def _static_tables():
    if "tables" in _CACHE:
        return _CACHE["tables"]
    perm = np.concatenate([np.arange(0, C, 2), np.arange(1, C, 2)])
    j = np.arange(H, dtype=np.float64)
    t = (T - 1 - np.arange(T)).astype(np.float64)
    ang = np.outer(j, t)                      # [H, T], angle of pair j at time t
    cosT = np.cos(ang).astype(ml_dtypes.bfloat16)
    sinT = np.sin(ang).astype(ml_dtypes.bfloat16)
    a = np.arange(PD)[:, None]
    b = np.arange(TCH)[None, :]
    masks = np.stack([(a + PD * d <= b) for d in range(NSP)])
    masks = masks.astype(ml_dtypes.bfloat16)
    ident = np.eye(PD, dtype=ml_dtypes.bfloat16)
    _CACHE["tables"] = (perm, cosT, sinT, masks, ident)
    return _CACHE["tables"]


def prepare(x, Wk, bk, Wq, bq, Wv, bv, Wp, bp):
    """Build (cached) the Bass program and the per-core input maps."""
    x = np.asarray(x, dtype=np.float32)
    Wk, bk = np.asarray(Wk, np.float32), np.asarray(bk, np.float32)
    Wq, bq = np.asarray(Wq, np.float32), np.asarray(bq, np.float32)
    Wv, bv = np.asarray(Wv, np.float32), np.asarray(bv, np.float32)
    Wp, bp = np.asarray(Wp, np.float32), np.asarray(bp, np.float32)

    perm, cosT, sinT, masks, ident = _static_tables()

    wk = np.ascontiguousarray(Wk[:, perm]).astype(ml_dtypes.bfloat16)
    wq = np.ascontiguousarray(Wq[:, perm]).astype(ml_dtypes.bfloat16)
    wvp = (Wv.astype(np.float64) @ Wp.astype(np.float64))
    bvp = (bv.astype(np.float64) @ Wp.astype(np.float64))
    wvp = wvp.astype(ml_dtypes.bfloat16)
    bvp = np.ascontiguousarray(bvp.reshape(1, C)).astype(np.float32)
    bkr = np.ascontiguousarray(bk[perm].reshape(NG, PD).T).astype(np.float32)
    bqr = np.ascontiguousarray(bq[perm].reshape(NG, PD).T).astype(np.float32)
    bpb = np.ascontiguousarray(np.broadcast_to(bp, (PD, C))).astype(np.float32)

    if "nc" not in _CACHE:
        _CACHE["nc"] = _build_nc()
    nc = _CACHE["nc"]

    shared = dict(wk=wk, wq=wq, wvp=wvp, bkr=bkr, bqr=bqr,
                  bvp=bvp, bpb=bpb, cosT=cosT, sinT=sinT, masks=masks,
                  ident=ident)
    xb = x.astype(ml_dtypes.bfloat16)
    in_maps = [dict(x=np.ascontiguousarray(xb[i]), **shared)
               for i in range(NCORES)]
    return nc, in_maps


def kernel(x, Wk, bk, Wq, bq, Wv, bv, Wp, bp):
    global LAST_RESULT
    nc, in_maps = prepare(x, Wk, bk, Wq, bq, Wv, bv, Wp, bp)
    res = run_bass_kernel_spmd(nc, in_maps, list(range(NCORES)))
    LAST_RESULT = res
    out = np.stack([res.results[i]["out"] for i in range(NCORES)], axis=0)
    return out.astype(np.float32)


# revision 39
# speedup vs baseline: 1.2275x; 1.0178x over previous
"""Trainium2 Bass kernel for nn_Attention_41996190220419.

Single-head causal attention with softplus weights and a time-flipped
rotary embedding, B=8 T=2048 C=1024 fp32.

Sharding: pure data-parallel over batch (1 batch element per NeuronCore,
8 cores, no collectives).

Optimizations over the 413us baseline (all per-core):
  * Wv/Wp folding: out = (wei @ x) @ (Wv Wp) + rowsum(wei) x (bv Wp) + bp.
    The V GEMM (256 N=512 matmuls) disappears; wei@x consumes x in its
    native [t, c] layout (the same tiles the transposes read), and the
    rank-1 rowsum term is PE-cheap: per span one M=1 accumulation chain
    over the masked score tiles plus one K=1 float32r matmul appended to
    each projection accumulation group.
  * Scores in fp8 e4m3 with MatmulPerfMode.DoubleRow (2 contraction rows
    per PE cell): the rotated K/Q are written by the DVE straight into
    [128, 2, T] paired-group tiles; each score tile is 2 halves x 4
    DoubleRow matmuls (K=256 each) instead of 8 bf16 N=512 matmuls.
    Numerically validated: L2 rel err ~6.6e-3 (budget 2e-2).
  * Causal skip: diagonal-block score halves that are fully above the
    diagonal (d>=2, left half) are never computed (gpsimd memset zeroes
    the st half instead).
  * Rotation arithmetic in bf16 (2x DVE throughput); error is far below
    the fp8 quantization already applied to the rotated K/Q.

Per-core phases (matmuls bf16 with fp32 PSUM accumulate unless noted):
  0: x [T,C] bf16 -> 16 resident xs tiles; XT via PE transposes
  1: KT/QT = W^T XT (+bias via ACT), bf16 rotation on DVE -> fp8 pair
     tiles kr8/qr8 [128, 2, T]
  2: per 512-span: ST[j,i] via fp8 DoubleRow (halves of 256), softplus =
     Ln(Exp(x)+1) on ACT, diagonal masks on DVE, rowsum via M=1 chain
  3: OT[c,i] = sum_j x[j,c] ST[j,i] (PSUM accumulate over j, bf16)
  4: OUT[t,:] = sum_c OT[c,t] Wvp[c,:] + rowsum*bvp (K=1 f32r matmul in
     the same PSUM group) + bp -> DRAM

The even/odd rotation pairs are tile-level structure: Wk/Wq columns (and
bk/bq) are host-permuted to [evens|odds]; scores are invariant to any
channel permutation applied to both K and Q. cos/sin tables, masks and
the identity are host-precomputed inputs.
"""

import os
import sys

if "/opt/trn_rl_repo" not in sys.path:
    sys.path.insert(0, "/opt/trn_rl_repo")

import numpy as np
import ml_dtypes

import concourse.bass as bass
import concourse.bacc as bacc
import concourse.mybir as mybir
import concourse.tile as tile
from concourse.bass_utils import run_bass_kernel_spmd

B, T, C = 8, 2048, 1024
H = C // 2
NCORES = 8
PD = 128
TCH = 512                 # t-chunk width (phase 1) == i-span width (attention)
HF = 256                  # fp8 DoubleRow moving half-width
NT = T // PD              # 16
NSP = T // TCH            # 4
NG = C // PD              # 8
NPR = NG // 2             # 4 fp8 pair tiles
BF16 = mybir.dt.bfloat16
F32 = mybir.dt.float32
F32R = mybir.dt.float32r
F8 = mybir.dt.float8e4
DR = mybir.MatmulPerfMode.DoubleRow
AF = mybir.ActivationFunctionType
INV_SQRT_C = float(C) ** -0.5

_CACHE = {}

LAST_RESULT = None  # BassKernelResults of the most recent run (for profiling)


def _patch_act_tables():
    """Force every ACT func we use (Copy/Identity/Exp/Ln) to resolve to the
    single `natural_log_exp_and_others` table so the Exp/Ln alternation in
    the softplus does not thrash ACT_TABLE_LOADs (1.3us each).
    Table ids are positional, so keep the dict order and only strip
    functions from the other tables."""
    if _CACHE.get("act_patched"):
        return
    from concourse import hw_specs
    orig = hw_specs.get_activation_tables
    combined = "natural_log_exp_and_others"

    def patched(arch):
        tables = orig(arch)
        if combined in tables:
            keep = tables[combined]
            tables = {
                name: (s if name == combined else (s - keep))
                for name, s in tables.items()
            }
        return tables

    hw_specs.get_activation_tables = patched
    bacc.get_activation_tables = patched
    _CACHE["act_patched"] = True


def _build_nc():
    _patch_act_tables()
    nc = bacc.Bacc("TRN2", target_bir_lowering=False, debug=False,
                   num_devices=NCORES)

    x_d = nc.dram_tensor("x", [T, C], BF16, kind="ExternalInput").ap()
    wk_d = nc.dram_tensor("wk", [C, C], BF16, kind="ExternalInput").ap()
    wq_d = nc.dram_tensor("wq", [C, C], BF16, kind="ExternalInput").ap()
    wvp_d = nc.dram_tensor("wvp", [C, C], BF16, kind="ExternalInput").ap()
    bkr_d = nc.dram_tensor("bkr", [PD, NG], F32, kind="ExternalInput").ap()
    bqr_d = nc.dram_tensor("bqr", [PD, NG], F32, kind="ExternalInput").ap()
    bvp_d = nc.dram_tensor("bvp", [1, C], F32R, kind="ExternalInput").ap()
    bpb_d = nc.dram_tensor("bpb", [PD, C], F32, kind="ExternalInput").ap()
    cos_d = nc.dram_tensor("cosT", [H, T], BF16, kind="ExternalInput").ap()
    sin_d = nc.dram_tensor("sinT", [H, T], BF16, kind="ExternalInput").ap()
    msk_d = nc.dram_tensor("masks", [NSP, PD, TCH], BF16,
                           kind="ExternalInput").ap()
    idn_d = nc.dram_tensor("ident", [PD, PD], BF16, kind="ExternalInput").ap()
    out_d = nc.dram_tensor("out", [T, C], F32, kind="ExternalOutput").ap()

    with tile.TileContext(nc) as tc:
        with tc.tile_pool(name="persist", bufs=1) as pp:

            # resident x in native [t, c] layout (transpose source + OT lhsT).
            # DMA queue engines run ~16 GB/s each, so big tiles are split
            # into chunks that land on different queues; the first two x
            # tiles (critical path for the transposes) are split finest.
            xs = [pp.tile([PD, C], BF16, tag=f"xs{j}", name=f"xs{j}")
                  for j in range(NT)]

            def load_xs(j, nchunk, eng=None):
                eng = eng or nc.sync
                w = C // nchunk
                for cc in range(nchunk):
                    eng.dma_start(
                        out=xs[j][:, cc * w:(cc + 1) * w],
                        in_=x_d[j * PD:(j + 1) * PD, cc * w:(cc + 1) * w])

            # first chunk's x tiles split finest and spread across all three
            # DMA rings so the transposes can start ~12us in
            load_xs(0, 4)
            ident = pp.tile([PD, PD], BF16, name="ident")
            nc.sync.dma_start(out=ident, in_=idn_d)
            load_xs(1, 4, nc.scalar)
            load_xs(2, 4, nc.gpsimd)
            load_xs(3, 4)
            # rotated K/Q fp8 pair tiles: slab 0 = even group e, slab 1 = odd
            # group e+4 (DoubleRow contracts both slabs per matmul)
            kr8 = [pp.tile([PD, 2, T], F8, tag=f"kr{e}", name=f"kr{e}")
                   for e in range(NPR)]
            qr8 = [pp.tile([PD, 2, T], F8, tag=f"qr{e}", name=f"qr{e}")
                   for e in range(NPR)]

            bkr = pp.tile([PD, NG], F32, name="bkr")
            nc.sync.dma_start(out=bkr, in_=bkr_d)
            bqr = pp.tile([PD, NG], F32, name="bqr")
            nc.sync.dma_start(out=bqr, in_=bqr_d)
            ones = pp.tile([PD, 1], BF16, name="ones")
            nc.gpsimd.memset(ones, 1.0)
            # everything below is needed late; dispatched from gpsimd's DMA
            # ring so the sync ring stays dedicated to x and trig
            mskt = []
            for d in range(NSP):
                m = pp.tile([PD, TCH], BF16, tag=f"msk{d}", name=f"msk{d}")
                mskt.append(m)
            bvp = pp.tile([1, C], F32R, name="bvp")
            bpb = pp.tile([PD, C], F32, name="bpb")
            # masked-score rowsums, one [1, TCH] f32 row per span
            rsum = [pp.tile([1, TCH], F32R, tag=f"rs{s}", name=f"rs{s}")
                    for s in range(NSP)]
            wpsb = [pp.tile([PD, C], BF16, tag=f"wp{ci}", name=f"wp{ci}")
                    for ci in range(NG)]

            def load_late_tensors():
                for ci in range(NG):
                    for cc in range(2):
                        nc.gpsimd.dma_start(
                            out=wpsb[ci][:, cc * 512:(cc + 1) * 512],
                            in_=wvp_d[ci * PD:(ci + 1) * PD,
                                      cc * 512:(cc + 1) * 512])
                for d in range(NSP):
                    nc.gpsimd.dma_start(out=mskt[d], in_=msk_d[d])
                nc.gpsimd.dma_start(out=bvp, in_=bvp_d)
                nc.gpsimd.dma_start(out=bpb, in_=bpb_d)

            # ---------------- phase 0 + 1: XT, rotated K/Q ----------------
            with tc.tile_pool(name="p1", bufs=1) as p1, \
                 tc.tile_pool(name="pstr", bufs=4, space="PSUM") as pstr, \
                 tc.tile_pool(name="psK", bufs=4, space="PSUM") as psK:
                # XT as one [128, group, t] tile so 4 transposes share one
                # PSUM tile and drain with a single wide copy
                xt = p1.tile([PD, NG, T], BF16, name="xt")

                # weight matrices prefetched up front, dispatch split across
                # the scalar and gpsimd DMA rings (sync is busy with x);
                # the late-phase tensors (wvp etc.) queue behind on gpsimd
                wsb = {}
                for wname, w_d in (("k", wk_d), ("q", wq_d)):
                    for ci in range(NG):
                        wt = p1.tile([PD, C], BF16, tag="w", bufs=16,
                                     name=f"w{wname}{ci}")
                        eng = nc.scalar if (wname, ci) < ("k", 4) else nc.gpsimd
                        for cc in range(2):
                            eng.dma_start(
                                out=wt[:, cc * 512:(cc + 1) * 512],
                                in_=w_d[ci * PD:(ci + 1) * PD,
                                        cc * 512:(cc + 1) * 512])
                        wsb[(wname, ci)] = wt
                load_late_tensors()

                # chunk-major: transpose the 4 t-blocks of chunk ch, then
                # run every K/Q chain for that chunk while the next chunk's
                # x tiles / trig stream in
                for ch in range(NSP):
                    trig = {}
                    for e in range(NPR):
                        csl = slice(ch * TCH, (ch + 1) * TCH)
                        cs = p1.tile([PD, TCH], BF16, tag="trig", bufs=10,
                                     name=f"cs{e}_{ch}")
                        nc.sync.dma_start(
                            out=cs, in_=cos_d[e * PD:(e + 1) * PD, csl])
                        sn = p1.tile([PD, TCH], BF16, tag="trig", bufs=10,
                                     name=f"sn{e}_{ch}")
                        nc.sync.dma_start(
                            out=sn, in_=sin_d[e * PD:(e + 1) * PD, csl])
                        trig[e] = (cs, sn)
                    if ch + 1 < NSP:
                        for j in range(4 * (ch + 1), 4 * (ch + 1) + 4):
                            load_xs(j, 2)

                    for j in range(4 * ch, 4 * ch + 4):
                        for half in range(2):
                            g0 = half * 4
                            ps = pstr.tile([PD, 4, PD], BF16, tag="ps_tr",
                                           name=f"ptr{j}_{half}")
                            for m in range(4):
                                g = g0 + m
                                nc.tensor.transpose(
                                    ps[:, m, :],
                                    xs[j][:, g * PD:(g + 1) * PD], ident)
                            dst = xt[:, g0:g0 + 4, j * PD:(j + 1) * PD]
                            if half == 0:
                                nc.scalar.activation(dst, ps, AF.Copy)
                            else:
                                nc.vector.tensor_copy(dst, ps)

                    for wname, brt, dst8 in (("k", bkr, kr8),
                                             ("q", bqr, qr8)):
                        for e in range(NPR):
                            o = e + NPR
                            tmp = {}
                            for g in (e, o):
                                ps = psK.tile([PD, TCH], F32, tag="ps_kq",
                                              name=f"pkq{wname}{g}_{ch}")
                                for ci in range(NG):
                                    nc.tensor.matmul(
                                        ps,
                                        lhsT=wsb[(wname, ci)][:,
                                                              g * PD:(g + 1) * PD],
                                        rhs=xt[:, ci, ch * TCH:(ch + 1) * TCH],
                                        start=(ci == 0), stop=(ci == NG - 1))
                                kt = p1.tile([PD, TCH], BF16, tag="kttmp",
                                             bufs=12, name=f"kt{wname}{g}_{ch}")
                                nc.scalar.activation(kt, ps, AF.Identity,
                                                     bias=brt[:, g:g + 1])
                                tmp[g] = kt
                            sl = slice(ch * TCH, (ch + 1) * TCH)
                            cs, sn = trig[e]
                            ze, zo = tmp[e], tmp[o]
                            t1 = p1.tile([PD, TCH], BF16, tag="rot", bufs=8,
                                         name=f"r1{wname}{e}_{ch}")
                            nc.vector.tensor_mul(t1, ze, cs)
                            t2 = p1.tile([PD, TCH], BF16, tag="rot", bufs=8,
                                         name=f"r2{wname}{e}_{ch}")
                            nc.vector.tensor_mul(t2, zo, sn)
                            nc.vector.tensor_add(dst8[e][:, 0, sl], t1, t2)
                            t3 = p1.tile([PD, TCH], BF16, tag="rot", bufs=8,
                                         name=f"r3{wname}{e}_{ch}")
                            nc.vector.tensor_mul(t3, zo, cs)
                            t4 = p1.tile([PD, TCH], BF16, tag="rot", bufs=8,
                                         name=f"r4{wname}{e}_{ch}")
                            nc.vector.tensor_mul(t4, ze, sn)
                            nc.vector.tensor_sub(dst8[e][:, 1, sl], t3, t4)

            # ---------------- phases 2-4: attention + projection ---------
            with tc.tile_pool(name="at", bufs=1) as at, \
                 tc.tile_pool(name="psS", bufs=3, space="PSUM") as psS, \
                 tc.tile_pool(name="psB", bufs=2, space="PSUM") as psB, \
                 tc.tile_pool(name="psP", bufs=2, space="PSUM") as psP:
                for s in range(NSP):
                    nj = 4 * (s + 1)
                    stact = []
                    for j in range(nj):
                        d = j - 4 * s
                        st = at.tile([PD, TCH], BF16, tag="stact", bufs=20,
                                     name=f"st{s}_{j}")
                        se = at.tile([PD, TCH], F32, tag="stexp", bufs=4,
                                     name=f"se{s}_{j}")
                        ps = psS.tile([PD, TCH], F32, tag="ps_sc",
                                      name=f"pst{s}_{j}")
                        h0 = 1 if d >= 2 else 0   # left half skip (causal)
                        for h in range(h0, 2):
                            for g in range(NPR):
                                nc.tensor.matmul(
                                    ps[:, h * HF:(h + 1) * HF],
                                    lhsT=qr8[g][:, :, j * PD:(j + 1) * PD],
                                    rhs=kr8[g][:, :,
                                               s * TCH + h * HF:
                                               s * TCH + (h + 1) * HF],
                                    start=(g == 0), stop=(g == NPR - 1),
                                    perf_mode=DR)
                        if h0:
                            # fully above the diagonal: never computed
                            nc.gpsimd.memset(st[:, :HF], 0.0)
                        asl = slice(h0 * HF, TCH)
                        # softplus(x) = ln(1 + exp(x)); scores/sqrt(C) are
                        # bounded to a few units so exp cannot overflow
                        nc.scalar.activation(se[:, asl], ps[:, asl], AF.Exp,
                                             scale=INV_SQRT_C)
                        nc.scalar.activation(st[:, asl], se[:, asl],
                                             AF.Ln, bias=1.0)
                        if d >= 0:
                            nc.vector.tensor_mul(st, st, mskt[d])
                        stact.append(st)

                    # masked-score rowsum for the rank-1 bv*Wp term: gpsimd
                    # (idle here) accumulates the j-tiles in f32, the last
                    # add casts to bf16, and the 128-partition reduction is a
                    # single M=1 matmul
                    acc = at.tile([PD, TCH], F32, tag="rsacc", bufs=2,
                                  name=f"rsacc{s}")
                    accb = at.tile([PD, TCH], BF16, tag="rsaccb", bufs=2,
                                   name=f"rsaccb{s}")
                    for j in range(nj):
                        if j == 0:
                            nc.gpsimd.tensor_copy(acc, stact[0])
                        elif j < nj - 1:
                            nc.gpsimd.tensor_add(acc, acc, stact[j])
                        else:
                            nc.gpsimd.tensor_add(accb, acc, stact[j])
                    psr = psS.tile([1, TCH], F32, tag="ps_rs", bufs=1,
                                   name=f"prs{s}")
                    nc.tensor.matmul(psr, lhsT=ones, rhs=accb,
                                     start=True, stop=True)
                    nc.scalar.activation(rsum[s], psr, AF.Copy)

                    ot = []
                    for g in range(NG):
                        ps2 = psB.tile([PD, TCH], F32, tag="ps_ot",
                                       name=f"pot{s}_{g}")
                        for j in range(nj):
                            nc.tensor.matmul(
                                ps2,
                                lhsT=xs[j][:, g * PD:(g + 1) * PD],
                                rhs=stact[j],
                                start=(j == 0), stop=(j == nj - 1))
                        o = at.tile([PD, TCH], BF16, tag="ot", bufs=16,
                                    name=f"ot{s}_{g}")
                        if g % 2 == 0:
                            nc.scalar.activation(o, ps2, AF.Copy)
                        else:
                            nc.vector.tensor_copy(o, ps2)
                        ot.append(o)

                    for tt in range(4):
                        trow = s * TCH + tt * PD
                        for h in range(2):
                            ps = psP.tile([PD, TCH], F32, tag="ps_mm",
                                          name=f"ppr{s}_{tt}_{h}")
                            for g in range(NG):
                                nc.tensor.matmul(
                                    ps,
                                    lhsT=ot[g][:, tt * PD:(tt + 1) * PD],
                                    rhs=wpsb[g][:, h * TCH:(h + 1) * TCH],
                                    start=(g == 0), stop=False)
                            # rank-1 rowsum x (bv Wp) joins the same PSUM
                            # accumulation group as a K=1 f32r matmul
                            nc.tensor.matmul(
                                ps,
                                lhsT=rsum[s][0:1, tt * PD:(tt + 1) * PD],
                                rhs=bvp[0:1, h * TCH:(h + 1) * TCH],
                                start=False, stop=True)
                            ob = at.tile([PD, TCH], F32, tag="ob", bufs=4,
                                         name=f"ob{s}_{tt}_{h}")
                            nc.vector.tensor_add(ob, ps,
                                                 bpb[:, h * TCH:(h + 1) * TCH])
                            for cc in range(2):
                                nc.sync.dma_start(
                                    out=out_d[trow:trow + PD,
                                              h * TCH + cc * HF:
                                              h * TCH + (cc + 1) * HF],
                                    in_=ob[:, cc * HF:(cc + 1) * HF])
    nc.finalize()
    return nc


def _static_tables():
    if "tables" in _CACHE:
        return _CACHE["tables"]
    perm = np.concatenate([np.arange(0, C, 2), np.arange(1, C, 2)])
    j = np.arange(H, dtype=np.float64)
    t = (T - 1 - np.arange(T)).astype(np.float64)
    ang = np.outer(j, t)                      # [H, T], angle of pair j at time t
    cosT = np.cos(ang).astype(ml_dtypes.bfloat16)
    sinT = np.sin(ang).astype(ml_dtypes.bfloat16)
    a = np.arange(PD)[:, None]
    b = np.arange(TCH)[None, :]
    masks = np.stack([(a + PD * d <= b) for d in range(NSP)])
    masks = masks.astype(ml_dtypes.bfloat16)
    ident = np.eye(PD, dtype=ml_dtypes.bfloat16)
    _CACHE["tables"] = (perm, cosT, sinT, masks, ident)
    return _CACHE["tables"]


def prepare(x, Wk, bk, Wq, bq, Wv, bv, Wp, bp):
    """Build (cached) the Bass program and the per-core input maps."""
    x = np.asarray(x, dtype=np.float32)
    Wk, bk = np.asarray(Wk, np.float32), np.asarray(bk, np.float32)
    Wq, bq = np.asarray(Wq, np.float32), np.asarray(bq, np.float32)
    Wv, bv = np.asarray(Wv, np.float32), np.asarray(bv, np.float32)
    Wp, bp = np.asarray(Wp, np.float32), np.asarray(bp, np.float32)

    perm, cosT, sinT, masks, ident = _static_tables()

    wk = np.ascontiguousarray(Wk[:, perm]).astype(ml_dtypes.bfloat16)
    wq = np.ascontiguousarray(Wq[:, perm]).astype(ml_dtypes.bfloat16)
    wvp = (Wv.astype(np.float64) @ Wp.astype(np.float64))
    bvp = (bv.astype(np.float64) @ Wp.astype(np.float64))
    wvp = wvp.astype(ml_dtypes.bfloat16)
    bvp = np.ascontiguousarray(bvp.reshape(1, C)).astype(np.float32)
    bkr = np.ascontiguousarray(bk[perm].reshape(NG, PD).T).astype(np.float32)
    bqr = np.ascontiguousarray(bq[perm].reshape(NG, PD).T).astype(np.float32)
    bpb = np.ascontiguousarray(np.broadcast_to(bp, (PD, C))).astype(np.float32)

    if "nc" not in _CACHE:
        _CACHE["nc"] = _build_nc()
    nc = _CACHE["nc"]

    shared = dict(wk=wk, wq=wq, wvp=wvp, bkr=bkr, bqr=bqr,
                  bvp=bvp, bpb=bpb, cosT=cosT, sinT=sinT, masks=masks,
                  ident=ident)
    xb = x.astype(ml_dtypes.bfloat16)
    in_maps = [dict(x=np.ascontiguousarray(xb[i]), **shared)
               for i in range(NCORES)]
    return nc, in_maps


def kernel(x, Wk, bk, Wq, bq, Wv, bv, Wp, bp):
    global LAST_RESULT
    nc, in_maps = prepare(x, Wk, bk, Wq, bq, Wv, bv, Wp, bp)
    res = run_bass_kernel_spmd(nc, in_maps, list(range(NCORES)))
    LAST_RESULT = res
    out = np.stack([res.results[i]["out"] for i in range(NCORES)], axis=0)
    return out.astype(np.float32)
